# revision 14
# baseline (speedup 1.0000x reference)
import sys

if "/opt/trn_rl_repo" not in sys.path:
    sys.path.insert(0, "/opt/trn_rl_repo")

import numpy as np
import ml_dtypes

import concourse.bass as bass
import concourse.tile as tile
from concourse import bacc, mybir
from concourse.bass_utils import run_bass_kernel_spmd
from concourse.masks import make_upper_triangular

F32 = mybir.dt.float32
BF16 = mybir.dt.bfloat16
BF16NP = ml_dtypes.bfloat16
EXP = mybir.ActivationFunctionType.Exp

# Problem shape (hardcoded per contract)
B, T, D = 4, 2048, 768
H, HD = 12, 64
N_CORES = 8
HPC = 6                  # heads per core
CPC = HPC * HD           # 384 qkv columns per core
TC = T // 128            # 16 token blocks
DC = D // 128            # 6 chunks of model dim
CC = CPC // 128          # 3 head-pair chunks per core
OC = D // 128            # 6 output-col chunks
WT = 512                 # PSUM f32 bank width
TW = T // WT             # 4 query quarters

# Query-quarter geometry: quarter q covers queries i in [q*WT,(q+1)*WT),
# attends key blocks jc in 0..QJHI[q]; block jc contributes columns
# i in [max(jc*128, q*WT), (q+1)*WT) => width QW[q][jc]; stored densely
# at offset QOFF[q][jc] in the per-(head,quarter) pT tile.
QJHI = [min(4 * q + 3, TC - 1) for q in range(TW)]
QW = [[WT - max(0, jc * 128 - q * WT) for jc in range(QJHI[q] + 1)] for q in range(TW)]
QOFF = []
for _q in range(TW):
    _off = [0]
    for _w in QW[_q]:
        _off.append(_off[-1] + _w)
    QOFF.append(_off)
QCOLS = [QOFF[q][-1] for q in range(TW)]
PTW = max(QCOLS)         # 7424

_CACHE = {}
CONFIG = {"fast_recip": True}


def _build_nc(fast_recip=True):
    nc = bacc.Bacc("TRN2", target_bir_lowering=False, debug=False)

    xT = nc.dram_tensor("xT", [D, T], BF16, kind="ExternalInput")
    wq = nc.dram_tensor("wq", [D, CPC], BF16, kind="ExternalInput")
    wk = nc.dram_tensor("wk", [D, CPC], BF16, kind="ExternalInput")
    wv = nc.dram_tensor("wv", [D, CPC], BF16, kind="ExternalInput")
    bq = nc.dram_tensor("bq", [128, CC], F32, kind="ExternalInput")
    bk = nc.dram_tensor("bk", [128, CC], F32, kind="ExternalInput")
    bv = nc.dram_tensor("bv", [128, CPC], F32, kind="ExternalInput")
    wo = nc.dram_tensor("wo", [CPC, D], BF16, kind="ExternalInput")
    yT = nc.dram_tensor("yT", [D, T], BF16, kind="ExternalOutput")

    with tile.TileContext(nc) as tc:
        with (
            tc.tile_pool(name="persist", bufs=1) as pp,
            tc.tile_pool(name="loadA", bufs=1) as pA,
            tc.tile_pool(name="bufs", bufs=1) as pBuf,
            tc.tile_pool(name="ps", bufs=1, space="PSUM") as psP,
        ):
            # chunk hc holds head 2hc on partitions 0:64, head 2hc+1 on 64:128
            qT_sb = pp.tile([128, CC, T], BF16)
            kT_sb = pp.tile([128, CC, T], BF16)
            # v with denominator-ones column, positioned so PV output lands
            # on the partitions the head's attnT half needs directly:
            #   even head: [v(0:64) | ones@64 | ...]   -> out rows 0:64 + l@64
            #   odd head:  [0(:32) | ones@32 | 0 | v(64:128)] -> l@32 + rows 64:128
            v_sb = pp.tile([128, TC, HPC, 128], BF16)
            attnT_sb = pp.tile([128, CC, T], BF16)
            wo_sb = pp.tile([128, CC, D], BF16)
            bq_sb = pp.tile([128, CC], F32)
            bk_sb = pp.tile([128, CC], F32)
            bv_sb = pp.tile([128, HPC, HD], F32)
            maskf = pp.tile([128, 128], F32)
            mask01 = pp.tile([128, 128], BF16)
            ones_sb = pp.tile([128, 128], BF16)

            xT_sb = pA.tile([128, DC, T], BF16)
            wq_sb = pA.tile([128, DC, CPC], BF16)
            wk_sb = pA.tile([128, DC, CPC], BF16)
            wv_sb = pA.tile([128, DC, CPC], BF16)

            # score megas: one 1024-col (2-bank) ring per head of the pair;
            # pair alternation on ACT gives effective double-buffering.
            me = psP.tile([128, 1024], F32, tag="me", bufs=1)
            mo = psP.tile([128, 1024], F32, tag="mo", bufs=1)

            # -------- DMAs in need-order --------
            xT_r = xT.ap().rearrange("(o p) t -> p o t", p=128)
            wq_r = wq.ap().rearrange("(o p) c -> p o c", p=128)
            wk_r = wk.ap().rearrange("(o p) c -> p o c", p=128)
            wv_r = wv.ap().rearrange("(o p) c -> p o c", p=128)
            nc.sync.dma_start(xT_sb[:, :, 0:WT], xT_r[:, :, 0:WT])
            nc.sync.dma_start(wq_sb[:, :, 0:128], wq_r[:, :, 0:128])
            nc.sync.dma_start(wk_sb[:, :, 0:128], wk_r[:, :, 0:128])
            nc.sync.dma_start(bq_sb[:], bq.ap())
            nc.sync.dma_start(bk_sb[:], bk.ap())
            nc.sync.dma_start(xT_sb[:, :, WT : 2 * WT], xT_r[:, :, WT : 2 * WT])
            nc.sync.dma_start(wv_sb[:], wv_r[:])
            nc.sync.dma_start(bv_sb[:], bv.ap())
            nc.sync.dma_start(wq_sb[:, :, 128:CPC], wq_r[:, :, 128:CPC])
            nc.sync.dma_start(wk_sb[:, :, 128:CPC], wk_r[:, :, 128:CPC])
            nc.sync.dma_start(xT_sb[:, :, 2 * WT : 3 * WT], xT_r[:, :, 2 * WT : 3 * WT])
            nc.sync.dma_start(xT_sb[:, :, 3 * WT : T], xT_r[:, :, 3 * WT : T])
            nc.sync.dma_start(wo_sb[:], wo.ap().rearrange("(c p) o -> p c o", p=128))

            # -------- init masks / ones / v zeros (off the critical path) ----
            # mask01[j, i] = 1.0 if j <= i else 0.0 (valid causal region)
            make_upper_triangular(nc, maskf, val=1.0, diag=True)
            nc.vector.tensor_copy(mask01[:], maskf[:])
            nc.gpsimd.memset(ones_sb[:], 1.0)
            # odd-head lhsT cols 0:64 must be zero except the ones col at 32
            nc.gpsimd.memset(v_sb[:, :, 1:HPC:2, 0:HD], 0.0)
            nc.gpsimd.memset(v_sb[:, :, 0:HPC:2, HD : HD + 1], 1.0)
            nc.gpsimd.memset(v_sb[:, :, 1:HPC:2, 32:33], 1.0)

            # -------- Phase A steps --------
            def qk_steps(hc, tw):
                sp = slice(tw * WT, (tw + 1) * WT)
                cs = slice(hc * 128, (hc + 1) * 128)

                def q_step(hc=hc, tw=tw):
                    ps_q = psP.tile(
                        [128, WT], F32, tag="work", bufs=3, name=f"psq{hc}_{tw}"
                    )
                    for di in range(DC):
                        nc.tensor.matmul(
                            ps_q[:],
                            wq_sb[:, di, cs],
                            xT_sb[:, di, sp],
                            start=(di == 0),
                            stop=(di == DC - 1),
                        )
                    nc.vector.tensor_scalar_add(
                        qT_sb[:, hc, sp], ps_q[:], bq_sb[:, hc : hc + 1]
                    )

                def k_step(hc=hc, tw=tw):
                    ps_k = psP.tile(
                        [128, WT], F32, tag="work", bufs=3, name=f"psk{hc}_{tw}"
                    )
                    for di in range(DC):
                        nc.tensor.matmul(
                            ps_k[:],
                            wk_sb[:, di, cs],
                            xT_sb[:, di, sp],
                            start=(di == 0),
                            stop=(di == DC - 1),
                        )
                    nc.vector.tensor_scalar_add(
                        kT_sb[:, hc, sp], ps_k[:], bk_sb[:, hc : hc + 1]
                    )

                return [(1400, q_step), (1400, k_step)]

            def v_step(tj):
                def step(tj=tj):
                    ps_v = psP.tile(
                        [128, HPC, HD], F32, tag="work", bufs=3, name=f"psv{tj}"
                    )
                    for di in range(DC):
                        nc.tensor.matmul(
                            ps_v[:],
                            xT_sb[:, di, tj * 128 : (tj + 1) * 128],
                            wv_sb[:, di, :],
                            start=(di == 0),
                            stop=(di == DC - 1),
                        )
                    nc.vector.tensor_add(
                        v_sb[:, tj, 0:HPC:2, 0:HD],
                        ps_v[:, 0:HPC:2, :],
                        bv_sb[:, 0:HPC:2, :],
                    )
                    nc.vector.tensor_add(
                        v_sb[:, tj, 1:HPC:2, HD:128],
                        ps_v[:, 1:HPC:2, :],
                        bv_sb[:, 1:HPC:2, :],
                    )

                return (1200, step)

            # -------- Phase B: paired scores + exp for (pair, quarter) ------
            def sq_steps(hcp, q, pT_e, pT_o):
                i0 = q * WT
                jhi = QJHI[q]
                # dense segment list: (jc, abs_lo_i, n, dense_col)
                segs = []
                pcol = 0
                for jc in range(jhi + 1):
                    w = QW[q][jc]
                    lo = i0 + (WT - w)
                    s0 = 0
                    while s0 < w:
                        n = min(WT - (pcol % WT), w - s0)
                        segs.append((jc, lo + s0, n, pcol))
                        s0 += n
                        pcol += n
                total = pcol
                # diagonal-block mask positions (dense col of block start)
                mask_pos = [QOFF[q][jc] for jc in range(4 * q, jhi + 1)]
                chunks = []
                cur, cbase = [], 0
                for s in segs:
                    cur.append(s)
                    cend = s[3] + s[2]
                    if cend - cbase == 1024 or cend == total:
                        chunks.append((cbase, cend, cur))
                        cur, cbase = [], cend

                for cb, ce, cseg in chunks:
                    def step(cb=cb, ce=ce, cseg=cseg):
                        for jc, lo, n, pc in cseg:
                            rp = pc % 1024
                            kb = slice(jc * 128, (jc + 1) * 128)
                            # paired row-tiled K=64 matmuls: even head on array
                            # rows 0:64, odd on 64:128 -> run concurrently
                            nc.tensor.matmul(
                                me[:, rp : rp + n],
                                kT_sb[0:64, hcp, kb],
                                qT_sb[0:64, hcp, lo : lo + n],
                                start=True,
                                stop=True,
                            )
                            nc.tensor.matmul(
                                mo[:, rp : rp + n],
                                kT_sb[64:128, hcp, kb],
                                qT_sb[64:128, hcp, lo : lo + n],
                                start=True,
                                stop=True,
                            )
                        rb = cb % 1024
                        w = ce - cb
                        nc.scalar.activation(pT_e[:, cb:ce], me[:, rb : rb + w], EXP)
                        nc.scalar.activation(pT_o[:, cb:ce], mo[:, rb : rb + w], EXP)
                        for mp in mask_pos:
                            if cb < mp + 128 <= ce:
                                nc.vector.tensor_mul(
                                    pT_e[:, mp : mp + 128],
                                    pT_e[:, mp : mp + 128],
                                    mask01[:],
                                )
                                nc.vector.tensor_mul(
                                    pT_o[:, mp : mp + 128],
                                    pT_o[:, mp : mp + 128],
                                    mask01[:],
                                )

                    yield (int((ce - cb) * 1.7) + 600, step)

            # -------- Phase B: PV + divide (+ W_o on last pair) -------------
            def pv_steps(hcp, q, pT_e, pT_o, emit_wo):
                i0 = q * WT
                jhi = QJHI[q]
                oTs, oUs, lPs, rcbs = {}, {}, {}, {}

                def groups(parity):
                    h = 2 * hcp + parity
                    pT = pT_e if parity == 0 else pT_o
                    mhi = HD + 1 if parity == 0 else 128
                    jcs = list(range(jhi + 1))
                    for g0 in range(0, len(jcs), 4):
                        grp = jcs[g0 : g0 + 4]

                        def step(grp=grp, g0=g0, h=h, pT=pT, mhi=mhi, parity=parity):
                            if g0 == 0:
                                oTs[parity] = psP.tile(
                                    [128, WT], F32, tag="work", bufs=3,
                                    name=f"oT{hcp}_{q}_{parity}",
                                )
                            oT = oTs[parity]
                            for jc in grp:
                                w = QW[q][jc]
                                nc.tensor.matmul(
                                    oT[0:mhi, WT - w : WT],
                                    v_sb[:, jc, h, 0:mhi],
                                    pT[:, QOFF[q][jc] : QOFF[q][jc] + w],
                                    start=(jc == 0),
                                    stop=(jc == jhi),
                                )

                        yield (len(grp) * 240, step)

                def div_chain(parity):
                    lrow = HD if parity == 0 else 32

                    def c_copy(parity=parity):
                        oU = pBuf.tile(
                            [128, WT], BF16, tag="oU", bufs=2,
                            name=f"oU{hcp}_{q}_{parity}",
                        )
                        oUs[parity] = oU
                        if parity == 0:
                            nc.vector.tensor_copy(
                                oU[0 : HD + 1, :], oTs[0][0 : HD + 1, :]
                            )
                        else:
                            # partition APs from base 32 may span <=32 rows,
                            # so the l row needs its own copy
                            nc.vector.tensor_copy(oU[HD:128, :], oTs[1][HD:128, :])
                            nc.vector.tensor_copy(oU[32:33, :], oTs[1][32:33, :])

                    def c_bcast(parity=parity, lrow=lrow):
                        # broadcast l to ALL 128 partitions so the custom DVE
                        # reciprocal runs at partition base 0
                        lP = psP.tile(
                            [128, WT], F32, tag="lP", bufs=1,
                            name=f"lP{hcp}_{q}_{parity}",
                        )
                        lPs[parity] = lP
                        nc.tensor.matmul(
                            lP[:, :],
                            ones_sb[lrow : lrow + 1, :],
                            oUs[parity][lrow : lrow + 1, :],
                            start=True,
                            stop=True,
                        )

                    def c_recip(parity=parity):
                        rcb = pBuf.tile(
                            [128, WT], F32, tag="rcb", bufs=2,
                            name=f"rcb{hcp}_{q}_{parity}",
                        )
                        rcbs[parity] = rcb
                        if fast_recip:
                            nc.vector.reciprocal_approx_fast(rcb[:, :], lPs[parity][:, :])
                        else:
                            nc.vector.reciprocal(rcb[:, :], lPs[parity][:, :])

                    def c_mul(parity=parity):
                        ob = 0 if parity == 0 else HD
                        nc.vector.tensor_mul(
                            attnT_sb[ob : ob + HD, hcp, i0 : i0 + WT],
                            oUs[parity][ob : ob + HD, :],
                            rcbs[parity][ob : ob + HD, :],
                        )

                    return [(660, c_copy), (250, c_bcast), (690, c_recip), (600, c_mul)]

                yield from groups(0)
                ediv = div_chain(0)
                yield ediv[0]
                og = list(groups(1))
                rest = ediv[1:]
                for i, s in enumerate(og):
                    yield s
                    if i < len(rest):
                        yield rest[i]
                for j in range(len(og), len(rest)):
                    yield rest[j]
                for s in div_chain(1):
                    yield s

                if emit_wo:
                    for oc in range(OC):
                        def dstep(oc=oc):
                            ps_wo = psP.tile(
                                [128, WT], F32, tag="work", bufs=3,
                                name=f"pswo{q}_{oc}",
                            )
                            for dc in range(CC):
                                nc.tensor.matmul(
                                    ps_wo[:],
                                    wo_sb[:, dc, oc * 128 : (oc + 1) * 128],
                                    attnT_sb[:, dc, i0 : i0 + WT],
                                    start=(dc == 0),
                                    stop=(dc == CC - 1),
                                )
                            ot = pBuf.tile(
                                [128, WT], BF16, tag="ot", bufs=2,
                                name=f"ot{q}_{oc}",
                            )
                            if oc % 2 == 0:
                                nc.scalar.copy(ot[:], ps_wo[:])
                            else:
                                nc.vector.tensor_copy(ot[:], ps_wo[:])
                            nc.sync.dma_start(
                                yT.ap()[oc * 128 : (oc + 1) * 128, i0 : i0 + WT],
                                ot[:],
                            )

                        yield (900, dstep)

            def interleave(a_steps, b_steps):
                """Emit steps from both streams, pacing by estimated cost."""
                a, b = list(a_steps), list(b_steps)
                ta = sum(c for c, _ in a) or 1
                tb = sum(c for c, _ in b) or 1
                ca = cb = 0.0
                ai = bi = 0
                while ai < len(a) or bi < len(b):
                    if bi < len(b) and (ai >= len(a) or cb * ta <= ca * tb):
                        cb += b[bi][0]
                        b[bi][1]()
                        bi += 1
                    else:
                        ca += a[ai][0]
                        a[ai][1]()
                        ai += 1

            # -------- emission schedule --------
            # A1: q/k for pair 0 token-quarter 0 only — scores(0,0) depends
            # on just this; the rest streams under the pair-0 score windows
            # so exp starts as early as possible
            for _c, fn in qk_steps(0, 0):
                fn()

            # A2: remaining projections, interleaved under pair-0 scores
            a2 = []
            for tw in range(1, TW):
                a2 += qk_steps(0, tw)
            for tj in range(4):
                a2.append(v_step(tj))
            for tw in range(TW):
                a2 += qk_steps(1, tw)
            for tj in range(4, 8):
                a2.append(v_step(tj))
            for tw in range(TW):
                a2 += qk_steps(2, tw)
            for tj in range(8, TC):
                a2.append(v_step(tj))

            # spread a2 across pair-0's four windows proportionally to the
            # score-stream cost, so the PE FIFO never head-of-line blocks
            # behind a score matmul that waits on exp
            sq_all = {
                (hcp, q): None for hcp in range(CC) for q in range(TW)
            }
            pT_tiles = {}
            for hcp in range(CC):
                for q in range(TW):
                    pT_e = pBuf.tile(
                        [128, PTW], BF16, tag="pT", bufs=4, name=f"pTe{hcp}_{q}"
                    )
                    pT_o = pBuf.tile(
                        [128, PTW], BF16, tag="pT", bufs=4, name=f"pTo{hcp}_{q}"
                    )
                    pT_tiles[(hcp, q)] = (pT_e, pT_o)

            sq0 = {q: list(sq_steps(0, q, *pT_tiles[(0, q)])) for q in range(TW)}
            sq0_costs = [sum(c for c, _ in sq0[q]) for q in range(TW)]
            total_sq0 = sum(sq0_costs)
            a2_total = sum(c for c, _ in a2)
            a2_slices = []
            acc = 0.0
            ai = 0
            for q in range(TW):
                acc += sq0_costs[q]
                target = a2_total * acc / total_sq0
                sl = []
                run = sum(c for c, _ in a2[:ai])
                while ai < len(a2) and run < target:
                    sl.append(a2[ai])
                    run += a2[ai][0]
                    ai += 1
                a2_slices.append(sl)
            a2_slices[-1] += a2[ai:]

            win_b = []
            for hcp in range(CC):
                for q in range(TW):
                    if hcp == 0:
                        # a2 slice BEFORE pv steps: emission order is the
                        # dependency contract (pv reads v_sb written by a2)
                        win_b = a2_slices[q] + win_b
                        sq = sq0[q]
                    else:
                        sq = list(sq_steps(hcp, q, *pT_tiles[(hcp, q)]))
                    interleave(sq, win_b)
                    win_b = list(
                        pv_steps(hcp, q, *pT_tiles[(hcp, q)], emit_wo=(hcp == CC - 1))
                    )
            for _c, fn in win_b:
                fn()

    nc.compile()
    return nc


def _get_nc():
    key = ("nc", CONFIG["fast_recip"])
    if key not in _CACHE:
        _CACHE[key] = _build_nc(CONFIG["fast_recip"])
    return _CACHE[key]


def kernel(x, W_qkv, b_qkv, W_o, b_o, **run_kwargs):
    x = np.asarray(x, dtype=np.float32)
    W_qkv = np.asarray(W_qkv, dtype=np.float32)
    b_qkv = np.asarray(b_qkv, dtype=np.float32)
    W_o = np.asarray(W_o, dtype=np.float32)
    b_o = np.asarray(b_o, dtype=np.float32)

    scale = np.float32(1.0) / np.sqrt(np.float32(HD)).astype(np.float32)

    in_maps = []
    for c in range(N_CORES):
        b = c // 2
        g = c % 2
        cs = g * CPC
        q_sl = slice(cs, cs + CPC)
        k_sl = slice(D + cs, D + cs + CPC)
        v_sl = slice(2 * D + cs, 2 * D + cs + CPC)
        in_maps.append(
            {
                "xT": np.ascontiguousarray(x[b].T).astype(BF16NP),
                "wq": (np.ascontiguousarray(W_qkv[:, q_sl]) * scale).astype(BF16NP),
                "wk": np.ascontiguousarray(W_qkv[:, k_sl]).astype(BF16NP),
                "wv": np.ascontiguousarray(W_qkv[:, v_sl]).astype(BF16NP),
                "bq": np.ascontiguousarray((b_qkv[q_sl] * scale).reshape(CC, 128).T),
                "bk": np.ascontiguousarray(b_qkv[k_sl].reshape(CC, 128).T),
                "bv": np.ascontiguousarray(np.broadcast_to(b_qkv[v_sl], (128, CPC))),
                "wo": np.ascontiguousarray(W_o[cs : cs + CPC, :]).astype(BF16NP),
            }
        )

    nc = _get_nc()
    res = run_bass_kernel_spmd(nc, in_maps, core_ids=list(range(N_CORES)), **run_kwargs)
    _CACHE["last_result"] = res

    out = np.empty((B, T, D), dtype=np.float32)
    for b in range(B):
        acc = res.results[2 * b]["yT"].astype(np.float32) + res.results[
            2 * b + 1
        ]["yT"].astype(np.float32)
        out[b] = acc.T + b_o
    return out


# revision 16
# speedup vs baseline: 1.0981x; 1.0981x over previous
import sys

if "/opt/trn_rl_repo" not in sys.path:
    sys.path.insert(0, "/opt/trn_rl_repo")

import numpy as np
import ml_dtypes

import concourse.bass as bass
import concourse.tile as tile
from concourse import bacc, mybir
from concourse.bass_utils import run_bass_kernel_spmd
from concourse.masks import make_upper_triangular

F32 = mybir.dt.float32
BF16 = mybir.dt.bfloat16
BF16NP = ml_dtypes.bfloat16
EXP = mybir.ActivationFunctionType.Exp

# Problem shape (hardcoded per contract)
B, T, D = 4, 2048, 768
H, HD = 12, 64
N_CORES = 8
HPC = 6                  # heads per core
CPC = HPC * HD           # 384 qkv columns per core
TC = T // 128            # 16 token blocks
DC = D // 128            # 6 chunks of model dim
CC = CPC // 128          # 3 head-pair chunks per core
OC = D // 128            # 6 output-col chunks
WT = 512                 # PSUM f32 bank width
TW = T // WT             # 4 query quarters

# Query-quarter geometry: quarter q covers queries i in [q*WT,(q+1)*WT),
# attends key blocks jc in 0..QJHI[q]; block jc contributes columns
# i in [max(jc*128, q*WT), (q+1)*WT) => width QW[q][jc]; stored densely
# at offset QOFF[q][jc] in the per-(head,quarter) pT tile.
QJHI = [min(4 * q + 3, TC - 1) for q in range(TW)]
QW = [[WT - max(0, jc * 128 - q * WT) for jc in range(QJHI[q] + 1)] for q in range(TW)]
QOFF = []
for _q in range(TW):
    _off = [0]
    for _w in QW[_q]:
        _off.append(_off[-1] + _w)
    QOFF.append(_off)
QCOLS = [QOFF[q][-1] for q in range(TW)]
PTW = max(QCOLS)         # 7424

_CACHE = {}
CONFIG = {"fast_recip": True}


def _build_nc(fast_recip=True):
    nc = bacc.Bacc("TRN2", target_bir_lowering=False, debug=False)

    xT = nc.dram_tensor("xT", [D, T], BF16, kind="ExternalInput")
    wq = nc.dram_tensor("wq", [D, CPC], BF16, kind="ExternalInput")
    wk = nc.dram_tensor("wk", [D, CPC], BF16, kind="ExternalInput")
    wv = nc.dram_tensor("wv", [D, CPC], BF16, kind="ExternalInput")
    bq = nc.dram_tensor("bq", [128, CC], F32, kind="ExternalInput")
    bk = nc.dram_tensor("bk", [128, CC], F32, kind="ExternalInput")
    bv = nc.dram_tensor("bv", [128, CPC], F32, kind="ExternalInput")
    wo = nc.dram_tensor("wo", [CPC, D], BF16, kind="ExternalInput")
    yT = nc.dram_tensor("yT", [D, T], BF16, kind="ExternalOutput")

    with tile.TileContext(nc) as tc:
        with (
            tc.tile_pool(name="persist", bufs=1) as pp,
            tc.tile_pool(name="loadA", bufs=1) as pA,
            tc.tile_pool(name="bufs", bufs=1) as pBuf,
            tc.tile_pool(name="ps", bufs=1, space="PSUM") as psP,
        ):
            # chunk hc holds head 2hc on partitions 0:64, head 2hc+1 on 64:128
            qT_sb = pp.tile([128, CC, T], BF16)
            kT_sb = pp.tile([128, CC, T], BF16)
            # v with denominator-ones column, positioned so PV output lands
            # on the partitions the head's attnT half needs directly:
            #   even head: [v(0:64) | ones@64 | ...]   -> out rows 0:64 + l@64
            #   odd head:  [0(:32) | ones@32 | 0 | v(64:128)] -> l@32 + rows 64:128
            v_sb = pp.tile([128, TC, HPC, 128], BF16)
            attnT_sb = pp.tile([128, CC, T], BF16)
            wo_sb = pp.tile([128, CC, D], BF16)
            bq_sb = pp.tile([128, CC], F32)
            bk_sb = pp.tile([128, CC], F32)
            bv_sb = pp.tile([128, HPC, HD], F32)
            maskf = pp.tile([128, 128], F32)
            mask01 = pp.tile([128, 128], BF16)
            ones_sb = pp.tile([128, 128], BF16)

            xT_sb = pA.tile([128, DC, T], BF16)
            wq_sb = pA.tile([128, DC, CPC], BF16)
            wk_sb = pA.tile([128, DC, CPC], BF16)
            wv_sb = pA.tile([128, DC, CPC], BF16)

            # score megas: one 1024-col (2-bank) ring per head of the pair;
            # pair alternation on ACT gives effective double-buffering.
            me = psP.tile([128, 1024], F32, tag="me", bufs=1)
            mo = psP.tile([128, 1024], F32, tag="mo", bufs=1)

            # -------- DMAs in need-order --------
            xT_r = xT.ap().rearrange("(o p) t -> p o t", p=128)
            wq_r = wq.ap().rearrange("(o p) c -> p o c", p=128)
            wk_r = wk.ap().rearrange("(o p) c -> p o c", p=128)
            wv_r = wv.ap().rearrange("(o p) c -> p o c", p=128)
            nc.sync.dma_start(xT_sb[:, :, 0:WT], xT_r[:, :, 0:WT])
            nc.sync.dma_start(wq_sb[:, :, 0:128], wq_r[:, :, 0:128])
            nc.sync.dma_start(wk_sb[:, :, 0:128], wk_r[:, :, 0:128])
            nc.sync.dma_start(bq_sb[:], bq.ap())
            nc.sync.dma_start(bk_sb[:], bk.ap())
            nc.sync.dma_start(xT_sb[:, :, WT : 2 * WT], xT_r[:, :, WT : 2 * WT])
            nc.sync.dma_start(wv_sb[:], wv_r[:])
            nc.sync.dma_start(bv_sb[:], bv.ap())
            nc.sync.dma_start(wq_sb[:, :, 128:CPC], wq_r[:, :, 128:CPC])
            nc.sync.dma_start(wk_sb[:, :, 128:CPC], wk_r[:, :, 128:CPC])
            nc.sync.dma_start(xT_sb[:, :, 2 * WT : 3 * WT], xT_r[:, :, 2 * WT : 3 * WT])
            nc.sync.dma_start(xT_sb[:, :, 3 * WT : T], xT_r[:, :, 3 * WT : T])
            nc.sync.dma_start(wo_sb[:], wo.ap().rearrange("(c p) o -> p c o", p=128))

            # -------- init masks / ones / v zeros (off the critical path) ----
            # mask01[j, i] = 1.0 if j <= i else 0.0 (valid causal region)
            make_upper_triangular(nc, maskf, val=1.0, diag=True)
            nc.vector.tensor_copy(mask01[:], maskf[:])
            nc.gpsimd.memset(ones_sb[:], 1.0)
            # odd-head lhsT cols 0:64 must be zero except the ones col at 32
            nc.gpsimd.memset(v_sb[:, :, 1:HPC:2, 0:HD], 0.0)
            nc.gpsimd.memset(v_sb[:, :, 0:HPC:2, HD : HD + 1], 1.0)
            nc.gpsimd.memset(v_sb[:, :, 1:HPC:2, 32:33], 1.0)

            # -------- Phase A steps --------
            def qk_steps(hc, tw):
                sp = slice(tw * WT, (tw + 1) * WT)
                cs = slice(hc * 128, (hc + 1) * 128)

                def q_step(hc=hc, tw=tw):
                    ps_q = psP.tile(
                        [128, WT], F32, tag="work", bufs=3, name=f"psq{hc}_{tw}"
                    )
                    for di in range(DC):
                        nc.tensor.matmul(
                            ps_q[:],
                            wq_sb[:, di, cs],
                            xT_sb[:, di, sp],
                            start=(di == 0),
                            stop=(di == DC - 1),
                        )
                    nc.vector.tensor_scalar_add(
                        qT_sb[:, hc, sp], ps_q[:], bq_sb[:, hc : hc + 1]
                    )

                def k_step(hc=hc, tw=tw):
                    ps_k = psP.tile(
                        [128, WT], F32, tag="work", bufs=3, name=f"psk{hc}_{tw}"
                    )
                    for di in range(DC):
                        nc.tensor.matmul(
                            ps_k[:],
                            wk_sb[:, di, cs],
                            xT_sb[:, di, sp],
                            start=(di == 0),
                            stop=(di == DC - 1),
                        )
                    nc.vector.tensor_scalar_add(
                        kT_sb[:, hc, sp], ps_k[:], bk_sb[:, hc : hc + 1]
                    )

                return [(1400, q_step), (1400, k_step)]

            def v_step(tj):
                def step(tj=tj):
                    ps_v = psP.tile(
                        [128, HPC, HD], F32, tag="work", bufs=3, name=f"psv{tj}"
                    )
                    for di in range(DC):
                        nc.tensor.matmul(
                            ps_v[:],
                            xT_sb[:, di, tj * 128 : (tj + 1) * 128],
                            wv_sb[:, di, :],
                            start=(di == 0),
                            stop=(di == DC - 1),
                        )
                    nc.vector.tensor_add(
                        v_sb[:, tj, 0:HPC:2, 0:HD],
                        ps_v[:, 0:HPC:2, :],
                        bv_sb[:, 0:HPC:2, :],
                    )
                    nc.vector.tensor_add(
                        v_sb[:, tj, 1:HPC:2, HD:128],
                        ps_v[:, 1:HPC:2, :],
                        bv_sb[:, 1:HPC:2, :],
                    )

                return (1200, step)

            # -------- Phase B: paired scores + exp for (pair, quarter) ------
            def sq_steps(hcp, q, pT_e, pT_o):
                i0 = q * WT
                jhi = QJHI[q]
                # dense segment list: (jc, abs_lo_i, n, dense_col)
                segs = []
                pcol = 0
                for jc in range(jhi + 1):
                    w = QW[q][jc]
                    lo = i0 + (WT - w)
                    s0 = 0
                    while s0 < w:
                        n = min(WT - (pcol % WT), w - s0)
                        segs.append((jc, lo + s0, n, pcol))
                        s0 += n
                        pcol += n
                total = pcol
                # diagonal-block mask positions (dense col of block start)
                mask_pos = [QOFF[q][jc] for jc in range(4 * q, jhi + 1)]
                chunks = []
                cur, cbase = [], 0
                for s in segs:
                    cur.append(s)
                    cend = s[3] + s[2]
                    if cend - cbase == 1024 or cend == total:
                        chunks.append((cbase, cend, cur))
                        cur, cbase = [], cend

                for cb, ce, cseg in chunks:
                    def step(cb=cb, ce=ce, cseg=cseg):
                        for jc, lo, n, pc in cseg:
                            rp = pc % 1024
                            kb = slice(jc * 128, (jc + 1) * 128)
                            # paired row-tiled K=64 matmuls: even head on array
                            # rows 0:64, odd on 64:128 -> run concurrently
                            nc.tensor.matmul(
                                me[:, rp : rp + n],
                                kT_sb[0:64, hcp, kb],
                                qT_sb[0:64, hcp, lo : lo + n],
                                start=True,
                                stop=True,
                            )
                            nc.tensor.matmul(
                                mo[:, rp : rp + n],
                                kT_sb[64:128, hcp, kb],
                                qT_sb[64:128, hcp, lo : lo + n],
                                start=True,
                                stop=True,
                            )
                        rb = cb % 1024
                        w = ce - cb
                        nc.scalar.activation(pT_e[:, cb:ce], me[:, rb : rb + w], EXP)
                        nc.scalar.activation(pT_o[:, cb:ce], mo[:, rb : rb + w], EXP)
                        for mp in mask_pos:
                            if cb < mp + 128 <= ce:
                                # diag-block masking on the otherwise-idle
                                # GPSIMD engine keeps the DVE free for the
                                # evac/div chains
                                nc.gpsimd.tensor_mul(
                                    pT_e[:, mp : mp + 128],
                                    pT_e[:, mp : mp + 128],
                                    mask01[:],
                                )
                                nc.gpsimd.tensor_mul(
                                    pT_o[:, mp : mp + 128],
                                    pT_o[:, mp : mp + 128],
                                    mask01[:],
                                )

                    yield (int((ce - cb) * 1.7) + 600, step)

            # -------- Phase B: PV + divide (+ W_o on last pair) -------------
            def pv_steps(hcp, q, pT_e, pT_o, emit_wo):
                i0 = q * WT
                jhi = QJHI[q]
                oTs, oUs, lPs, rcbs = {}, {}, {}, {}

                def groups(parity):
                    h = 2 * hcp + parity
                    pT = pT_e if parity == 0 else pT_o
                    mhi = HD + 1 if parity == 0 else 128
                    jcs = list(range(jhi + 1))
                    for g0 in range(0, len(jcs), 4):
                        grp = jcs[g0 : g0 + 4]

                        def step(grp=grp, g0=g0, h=h, pT=pT, mhi=mhi, parity=parity):
                            if g0 == 0:
                                oTs[parity] = psP.tile(
                                    [128, WT], F32, tag="work", bufs=3,
                                    name=f"oT{hcp}_{q}_{parity}",
                                )
                            oT = oTs[parity]
                            for jc in grp:
                                w = QW[q][jc]
                                nc.tensor.matmul(
                                    oT[0:mhi, WT - w : WT],
                                    v_sb[:, jc, h, 0:mhi],
                                    pT[:, QOFF[q][jc] : QOFF[q][jc] + w],
                                    start=(jc == 0),
                                    stop=(jc == jhi),
                                )

                        yield (len(grp) * 240, step)

                def div_chain(parity):
                    lrow = HD if parity == 0 else 32

                    def c_copy(parity=parity):
                        oU = pBuf.tile(
                            [128, WT], BF16, tag="oU", bufs=2,
                            name=f"oU{hcp}_{q}_{parity}",
                        )
                        oUs[parity] = oU
                        if parity == 0:
                            nc.vector.tensor_copy(
                                oU[0 : HD + 1, :], oTs[0][0 : HD + 1, :]
                            )
                        else:
                            # partition APs from base 32 may span <=32 rows,
                            # so the l row needs its own copy
                            nc.vector.tensor_copy(oU[HD:128, :], oTs[1][HD:128, :])
                            nc.vector.tensor_copy(oU[32:33, :], oTs[1][32:33, :])

                    def c_bcast(parity=parity, lrow=lrow):
                        # broadcast l to ALL 128 partitions so the custom DVE
                        # reciprocal runs at partition base 0
                        lP = psP.tile(
                            [128, WT], F32, tag="lP", bufs=1,
                            name=f"lP{hcp}_{q}_{parity}",
                        )
                        lPs[parity] = lP
                        nc.tensor.matmul(
                            lP[:, :],
                            ones_sb[lrow : lrow + 1, :],
                            oUs[parity][lrow : lrow + 1, :],
                            start=True,
                            stop=True,
                        )

                    def c_recip(parity=parity):
                        rcb = pBuf.tile(
                            [128, WT], F32, tag="rcb", bufs=2,
                            name=f"rcb{hcp}_{q}_{parity}",
                        )
                        rcbs[parity] = rcb
                        if fast_recip:
                            nc.vector.reciprocal_approx_fast(rcb[:, :], lPs[parity][:, :])
                        else:
                            nc.vector.reciprocal(rcb[:, :], lPs[parity][:, :])

                    def c_mul(parity=parity):
                        ob = 0 if parity == 0 else HD
                        nc.vector.tensor_mul(
                            attnT_sb[ob : ob + HD, hcp, i0 : i0 + WT],
                            oUs[parity][ob : ob + HD, :],
                            rcbs[parity][ob : ob + HD, :],
                        )

                    return [(660, c_copy), (250, c_bcast), (690, c_recip), (600, c_mul)]

                yield from groups(0)
                ediv = div_chain(0)
                yield ediv[0]
                og = list(groups(1))
                rest = ediv[1:]
                for i, s in enumerate(og):
                    yield s
                    if i < len(rest):
                        yield rest[i]
                for j in range(len(og), len(rest)):
                    yield rest[j]
                for s in div_chain(1):
                    yield s

                if emit_wo:
                    for oc in range(OC):
                        def dstep(oc=oc):
                            ps_wo = psP.tile(
                                [128, WT], F32, tag="work", bufs=3,
                                name=f"pswo{q}_{oc}",
                            )
                            for dc in range(CC):
                                nc.tensor.matmul(
                                    ps_wo[:],
                                    wo_sb[:, dc, oc * 128 : (oc + 1) * 128],
                                    attnT_sb[:, dc, i0 : i0 + WT],
                                    start=(dc == 0),
                                    stop=(dc == CC - 1),
                                )
                            ot = pBuf.tile(
                                [128, WT], BF16, tag="ot", bufs=2,
                                name=f"ot{q}_{oc}",
                            )
                            if oc % 2 == 0:
                                nc.scalar.copy(ot[:], ps_wo[:])
                            else:
                                nc.vector.tensor_copy(ot[:], ps_wo[:])
                            nc.sync.dma_start(
                                yT.ap()[oc * 128 : (oc + 1) * 128, i0 : i0 + WT],
                                ot[:],
                            )

                        yield (900, dstep)

            def interleave(a_steps, b_steps):
                """Emit steps from both streams, pacing by estimated cost."""
                a, b = list(a_steps), list(b_steps)
                ta = sum(c for c, _ in a) or 1
                tb = sum(c for c, _ in b) or 1
                ca = cb = 0.0
                ai = bi = 0
                while ai < len(a) or bi < len(b):
                    if bi < len(b) and (ai >= len(a) or cb * ta <= ca * tb):
                        cb += b[bi][0]
                        b[bi][1]()
                        bi += 1
                    else:
                        ca += a[ai][0]
                        a[ai][1]()
                        ai += 1

            # -------- emission schedule --------
            # A1: q/k for pair 0 (runs immediately; keeps the PE queue deep
            # enough that HAM stays warm through the pipeline ramp)
            for tw in range(TW):
                for _c, fn in qk_steps(0, tw):
                    fn()

            # A2: remaining projections, interleaved under pair-0 scores
            a2 = []
            for tj in range(4):
                a2.append(v_step(tj))
            for tw in range(TW):
                a2 += qk_steps(1, tw)
            for tj in range(4, 8):
                a2.append(v_step(tj))
            for tw in range(TW):
                a2 += qk_steps(2, tw)
            for tj in range(8, TC):
                a2.append(v_step(tj))

            # spread a2 across pair-0's four windows proportionally to the
            # score-stream cost, so the PE FIFO never head-of-line blocks
            # behind a score matmul that waits on exp
            sq_all = {
                (hcp, q): None for hcp in range(CC) for q in range(TW)
            }
            pT_tiles = {}
            for hcp in range(CC):
                for q in range(TW):
                    pT_e = pBuf.tile(
                        [128, PTW], BF16, tag="pT", bufs=4, name=f"pTe{hcp}_{q}"
                    )
                    pT_o = pBuf.tile(
                        [128, PTW], BF16, tag="pT", bufs=4, name=f"pTo{hcp}_{q}"
                    )
                    pT_tiles[(hcp, q)] = (pT_e, pT_o)

            sq0 = {q: list(sq_steps(0, q, *pT_tiles[(0, q)])) for q in range(TW)}
            sq0_costs = [sum(c for c, _ in sq0[q]) for q in range(TW)]
            total_sq0 = sum(sq0_costs)
            a2_total = sum(c for c, _ in a2)
            a2_slices = []
            acc = 0.0
            ai = 0
            for q in range(TW):
                acc += sq0_costs[q]
                target = a2_total * acc / total_sq0
                sl = []
                run = sum(c for c, _ in a2[:ai])
                while ai < len(a2) and run < target:
                    sl.append(a2[ai])
                    run += a2[ai][0]
                    ai += 1
                a2_slices.append(sl)
            a2_slices[-1] += a2[ai:]

            win_b = []
            for hcp in range(CC):
                for q in range(TW):
                    if hcp == 0:
                        # a2 slice BEFORE pv steps: emission order is the
                        # dependency contract (pv reads v_sb written by a2)
                        win_b = a2_slices[q] + win_b
                        sq = sq0[q]
                    else:
                        sq = list(sq_steps(hcp, q, *pT_tiles[(hcp, q)]))
                    interleave(sq, win_b)
                    win_b = list(
                        pv_steps(hcp, q, *pT_tiles[(hcp, q)], emit_wo=(hcp == CC - 1))
                    )
            for _c, fn in win_b:
                fn()

    nc.compile()
    return nc


def _get_nc():
    key = ("nc", CONFIG["fast_recip"])
    if key not in _CACHE:
        _CACHE[key] = _build_nc(CONFIG["fast_recip"])
    return _CACHE[key]


def kernel(x, W_qkv, b_qkv, W_o, b_o, **run_kwargs):
    x = np.asarray(x, dtype=np.float32)
    W_qkv = np.asarray(W_qkv, dtype=np.float32)
    b_qkv = np.asarray(b_qkv, dtype=np.float32)
    W_o = np.asarray(W_o, dtype=np.float32)
    b_o = np.asarray(b_o, dtype=np.float32)

    scale = np.float32(1.0) / np.sqrt(np.float32(HD)).astype(np.float32)

    in_maps = []
    for c in range(N_CORES):
        b = c // 2
        g = c % 2
        cs = g * CPC
        q_sl = slice(cs, cs + CPC)
        k_sl = slice(D + cs, D + cs + CPC)
        v_sl = slice(2 * D + cs, 2 * D + cs + CPC)
        in_maps.append(
            {
                "xT": np.ascontiguousarray(x[b].T).astype(BF16NP),
                "wq": (np.ascontiguousarray(W_qkv[:, q_sl]) * scale).astype(BF16NP),
                "wk": np.ascontiguousarray(W_qkv[:, k_sl]).astype(BF16NP),
                "wv": np.ascontiguousarray(W_qkv[:, v_sl]).astype(BF16NP),
                "bq": np.ascontiguousarray((b_qkv[q_sl] * scale).reshape(CC, 128).T),
                "bk": np.ascontiguousarray(b_qkv[k_sl].reshape(CC, 128).T),
                "bv": np.ascontiguousarray(np.broadcast_to(b_qkv[v_sl], (128, CPC))),
                "wo": np.ascontiguousarray(W_o[cs : cs + CPC, :]).astype(BF16NP),
            }
        )

    nc = _get_nc()
    res = run_bass_kernel_spmd(nc, in_maps, core_ids=list(range(N_CORES)), **run_kwargs)
    _CACHE["last_result"] = res

    out = np.empty((B, T, D), dtype=np.float32)
    for b in range(B):
        acc = res.results[2 * b]["yT"].astype(np.float32) + res.results[
            2 * b + 1
        ]["yT"].astype(np.float32)
        out[b] = acc.T + b_o
    return out


# revision 17
# speedup vs baseline: 1.1044x; 1.0057x over previous
import sys

if "/opt/trn_rl_repo" not in sys.path:
    sys.path.insert(0, "/opt/trn_rl_repo")

import numpy as np
import ml_dtypes

import concourse.bass as bass
import concourse.tile as tile
from concourse import bacc, mybir
from concourse.bass_utils import run_bass_kernel_spmd
from concourse.masks import make_upper_triangular

F32 = mybir.dt.float32
BF16 = mybir.dt.bfloat16
BF16NP = ml_dtypes.bfloat16
EXP = mybir.ActivationFunctionType.Exp

# Problem shape (hardcoded per contract)
B, T, D = 4, 2048, 768
H, HD = 12, 64
N_CORES = 8
HPC = 6                  # heads per core
CPC = HPC * HD           # 384 qkv columns per core
TC = T // 128            # 16 token blocks
DC = D // 128            # 6 chunks of model dim
CC = CPC // 128          # 3 head-pair chunks per core
OC = D // 128            # 6 output-col chunks
WT = 512                 # PSUM f32 bank width
TW = T // WT             # 4 query quarters

# Query-quarter geometry: quarter q covers queries i in [q*WT,(q+1)*WT),
# attends key blocks jc in 0..QJHI[q]; block jc contributes columns
# i in [max(jc*128, q*WT), (q+1)*WT) => width QW[q][jc]; stored densely
# at offset QOFF[q][jc] in the per-(head,quarter) pT tile.
QJHI = [min(4 * q + 3, TC - 1) for q in range(TW)]
QW = [[WT - max(0, jc * 128 - q * WT) for jc in range(QJHI[q] + 1)] for q in range(TW)]
QOFF = []
for _q in range(TW):
    _off = [0]
    for _w in QW[_q]:
        _off.append(_off[-1] + _w)
    QOFF.append(_off)
QCOLS = [QOFF[q][-1] for q in range(TW)]
PTW = max(QCOLS)         # 7424

_CACHE = {}
CONFIG = {"fast_recip": True}


def _build_nc(fast_recip=True):
    nc = bacc.Bacc("TRN2", target_bir_lowering=False, debug=False)

    xT = nc.dram_tensor("xT", [D, T], BF16, kind="ExternalInput")
    wq = nc.dram_tensor("wq", [D, CPC], BF16, kind="ExternalInput")
    wk = nc.dram_tensor("wk", [D, CPC], BF16, kind="ExternalInput")
    wv = nc.dram_tensor("wv", [D, CPC], BF16, kind="ExternalInput")
    bq = nc.dram_tensor("bq", [128, CC], F32, kind="ExternalInput")
    bk = nc.dram_tensor("bk", [128, CC], F32, kind="ExternalInput")
    bv = nc.dram_tensor("bv", [128, CPC], F32, kind="ExternalInput")
    wo = nc.dram_tensor("wo", [CPC, D], BF16, kind="ExternalInput")
    yT = nc.dram_tensor("yT", [D, T], BF16, kind="ExternalOutput")

    with tile.TileContext(nc) as tc:
        with (
            tc.tile_pool(name="persist", bufs=1) as pp,
            tc.tile_pool(name="loadA", bufs=1) as pA,
            tc.tile_pool(name="bufs", bufs=1) as pBuf,
            tc.tile_pool(name="ps", bufs=1, space="PSUM") as psP,
        ):
            # chunk hc holds head 2hc on partitions 0:64, head 2hc+1 on 64:128
            qT_sb = pp.tile([128, CC, T], BF16)
            kT_sb = pp.tile([128, CC, T], BF16)
            # v with denominator-ones column, positioned so PV output lands
            # on the partitions the head's attnT half needs directly:
            #   even head: [v(0:64) | ones@64 | ...]   -> out rows 0:64 + l@64
            #   odd head:  [0(:32) | ones@32 | 0 | v(64:128)] -> l@32 + rows 64:128
            v_sb = pp.tile([128, TC, HPC, 128], BF16)
            attnT_sb = pp.tile([128, CC, T], BF16)
            wo_sb = pp.tile([128, CC, D], BF16)
            bq_sb = pp.tile([128, CC], F32)
            bk_sb = pp.tile([128, CC], F32)
            bv_sb = pp.tile([128, HPC, HD], F32)
            maskf = pp.tile([128, 128], F32)
            mask01 = pp.tile([128, 128], BF16)
            ones_sb = pp.tile([128, 128], BF16)

            xT_sb = pA.tile([128, DC, T], BF16)
            wq_sb = pA.tile([128, DC, CPC], BF16)
            wk_sb = pA.tile([128, DC, CPC], BF16)
            wv_sb = pA.tile([128, DC, CPC], BF16)

            # score megas: one 1024-col (2-bank) ring per head of the pair;
            # pair alternation on ACT gives effective double-buffering.
            me = psP.tile([128, 1024], F32, tag="me", bufs=1)
            mo = psP.tile([128, 1024], F32, tag="mo", bufs=1)

            # -------- DMAs in need-order --------
            xT_r = xT.ap().rearrange("(o p) t -> p o t", p=128)
            wq_r = wq.ap().rearrange("(o p) c -> p o c", p=128)
            wk_r = wk.ap().rearrange("(o p) c -> p o c", p=128)
            wv_r = wv.ap().rearrange("(o p) c -> p o c", p=128)
            nc.sync.dma_start(xT_sb[:, :, 0:WT], xT_r[:, :, 0:WT])
            nc.sync.dma_start(wq_sb[:, :, 0:128], wq_r[:, :, 0:128])
            nc.sync.dma_start(wk_sb[:, :, 0:128], wk_r[:, :, 0:128])
            nc.sync.dma_start(bq_sb[:], bq.ap())
            nc.sync.dma_start(bk_sb[:], bk.ap())
            nc.sync.dma_start(xT_sb[:, :, WT : 2 * WT], xT_r[:, :, WT : 2 * WT])
            nc.sync.dma_start(wv_sb[:], wv_r[:])
            nc.sync.dma_start(bv_sb[:], bv.ap())
            nc.sync.dma_start(wq_sb[:, :, 128:CPC], wq_r[:, :, 128:CPC])
            nc.sync.dma_start(wk_sb[:, :, 128:CPC], wk_r[:, :, 128:CPC])
            nc.sync.dma_start(xT_sb[:, :, 2 * WT : 3 * WT], xT_r[:, :, 2 * WT : 3 * WT])
            nc.sync.dma_start(xT_sb[:, :, 3 * WT : T], xT_r[:, :, 3 * WT : T])
            nc.sync.dma_start(wo_sb[:], wo.ap().rearrange("(c p) o -> p c o", p=128))

            # -------- init masks / ones / v zeros (off the critical path) ----
            # mask01[j, i] = 1.0 if j <= i else 0.0 (valid causal region)
            make_upper_triangular(nc, maskf, val=1.0, diag=True)
            nc.vector.tensor_copy(mask01[:], maskf[:])
            nc.gpsimd.memset(ones_sb[:], 1.0)
            # odd-head lhsT cols 0:64 must be zero except the ones col at 32
            nc.gpsimd.memset(v_sb[:, :, 1:HPC:2, 0:HD], 0.0)
            nc.gpsimd.memset(v_sb[:, :, 0:HPC:2, HD : HD + 1], 1.0)
            nc.gpsimd.memset(v_sb[:, :, 1:HPC:2, 32:33], 1.0)

            # -------- Phase A steps --------
            def qk_steps(hc, tw):
                sp = slice(tw * WT, (tw + 1) * WT)
                cs = slice(hc * 128, (hc + 1) * 128)

                def q_step(hc=hc, tw=tw):
                    ps_q = psP.tile(
                        [128, WT], F32, tag="work", bufs=3, name=f"psq{hc}_{tw}"
                    )
                    for di in range(DC):
                        nc.tensor.matmul(
                            ps_q[:],
                            wq_sb[:, di, cs],
                            xT_sb[:, di, sp],
                            start=(di == 0),
                            stop=(di == DC - 1),
                        )
                    nc.vector.tensor_scalar_add(
                        qT_sb[:, hc, sp], ps_q[:], bq_sb[:, hc : hc + 1]
                    )

                def k_step(hc=hc, tw=tw):
                    ps_k = psP.tile(
                        [128, WT], F32, tag="work", bufs=3, name=f"psk{hc}_{tw}"
                    )
                    for di in range(DC):
                        nc.tensor.matmul(
                            ps_k[:],
                            wk_sb[:, di, cs],
                            xT_sb[:, di, sp],
                            start=(di == 0),
                            stop=(di == DC - 1),
                        )
                    nc.vector.tensor_scalar_add(
                        kT_sb[:, hc, sp], ps_k[:], bk_sb[:, hc : hc + 1]
                    )

                return [(1400, q_step), (1400, k_step)]

            def v_step(tj):
                def step(tj=tj):
                    ps_v = psP.tile(
                        [128, HPC, HD], F32, tag="work", bufs=3, name=f"psv{tj}"
                    )
                    for di in range(DC):
                        nc.tensor.matmul(
                            ps_v[:],
                            xT_sb[:, di, tj * 128 : (tj + 1) * 128],
                            wv_sb[:, di, :],
                            start=(di == 0),
                            stop=(di == DC - 1),
                        )
                    nc.vector.tensor_add(
                        v_sb[:, tj, 0:HPC:2, 0:HD],
                        ps_v[:, 0:HPC:2, :],
                        bv_sb[:, 0:HPC:2, :],
                    )
                    nc.vector.tensor_add(
                        v_sb[:, tj, 1:HPC:2, HD:128],
                        ps_v[:, 1:HPC:2, :],
                        bv_sb[:, 1:HPC:2, :],
                    )

                return (1200, step)

            # -------- Phase B: paired scores + exp for (pair, quarter) ------
            def sq_steps(hcp, q, pT_e, pT_o):
                i0 = q * WT
                jhi = QJHI[q]
                # dense segment list: (jc, abs_lo_i, n, dense_col)
                segs = []
                pcol = 0
                for jc in range(jhi + 1):
                    w = QW[q][jc]
                    lo = i0 + (WT - w)
                    s0 = 0
                    while s0 < w:
                        n = min(WT - (pcol % WT), w - s0)
                        segs.append((jc, lo + s0, n, pcol))
                        s0 += n
                        pcol += n
                total = pcol
                # diagonal-block mask positions (dense col of block start)
                mask_pos = [QOFF[q][jc] for jc in range(4 * q, jhi + 1)]
                chunks = []
                cur, cbase = [], 0
                for s in segs:
                    cur.append(s)
                    cend = s[3] + s[2]
                    if cend - cbase == 1024 or cend == total:
                        chunks.append((cbase, cend, cur))
                        cur, cbase = [], cend

                for cb, ce, cseg in chunks:
                    def step(cb=cb, ce=ce, cseg=cseg):
                        for jc, lo, n, pc in cseg:
                            rp = pc % 1024
                            kb = slice(jc * 128, (jc + 1) * 128)
                            # paired row-tiled K=64 matmuls: even head on array
                            # rows 0:64, odd on 64:128 -> run concurrently
                            nc.tensor.matmul(
                                me[:, rp : rp + n],
                                kT_sb[0:64, hcp, kb],
                                qT_sb[0:64, hcp, lo : lo + n],
                                start=True,
                                stop=True,
                            )
                            nc.tensor.matmul(
                                mo[:, rp : rp + n],
                                kT_sb[64:128, hcp, kb],
                                qT_sb[64:128, hcp, lo : lo + n],
                                start=True,
                                stop=True,
                            )
                        rb = cb % 1024
                        w = ce - cb
                        nc.scalar.activation(pT_e[:, cb:ce], me[:, rb : rb + w], EXP)
                        nc.scalar.activation(pT_o[:, cb:ce], mo[:, rb : rb + w], EXP)
                        for mp in mask_pos:
                            if cb < mp + 128 <= ce:
                                # diag-block masking on the otherwise-idle
                                # GPSIMD engine keeps the DVE free for the
                                # evac/div chains
                                nc.gpsimd.tensor_mul(
                                    pT_e[:, mp : mp + 128],
                                    pT_e[:, mp : mp + 128],
                                    mask01[:],
                                )
                                nc.gpsimd.tensor_mul(
                                    pT_o[:, mp : mp + 128],
                                    pT_o[:, mp : mp + 128],
                                    mask01[:],
                                )

                    yield (int((ce - cb) * 1.7) + 600, step)

            # -------- Phase B: PV + divide (+ W_o on last pair) -------------
            def pv_steps(hcp, q, pT_e, pT_o, emit_wo):
                i0 = q * WT
                jhi = QJHI[q]
                oTs, oUs, lPs, rcbs = {}, {}, {}, {}

                def groups(parity):
                    h = 2 * hcp + parity
                    pT = pT_e if parity == 0 else pT_o
                    mhi = HD + 1 if parity == 0 else 128
                    jcs = list(range(jhi + 1))
                    for g0 in range(0, len(jcs), 4):
                        grp = jcs[g0 : g0 + 4]

                        def step(grp=grp, g0=g0, h=h, pT=pT, mhi=mhi, parity=parity):
                            if g0 == 0:
                                oTs[parity] = psP.tile(
                                    [128, WT], F32, tag="work", bufs=3,
                                    name=f"oT{hcp}_{q}_{parity}",
                                )
                            oT = oTs[parity]
                            for jc in grp:
                                w = QW[q][jc]
                                nc.tensor.matmul(
                                    oT[0:mhi, WT - w : WT],
                                    v_sb[:, jc, h, 0:mhi],
                                    pT[:, QOFF[q][jc] : QOFF[q][jc] + w],
                                    start=(jc == 0),
                                    stop=(jc == jhi),
                                )

                        yield (len(grp) * 240, step)

                def div_chain(parity):
                    lrow = HD if parity == 0 else 32

                    def c_copy(parity=parity):
                        oU = pBuf.tile(
                            [128, WT], BF16, tag="oU", bufs=2,
                            name=f"oU{hcp}_{q}_{parity}",
                        )
                        oUs[parity] = oU
                        if parity == 0:
                            nc.vector.tensor_copy(
                                oU[0 : HD + 1, :], oTs[0][0 : HD + 1, :]
                            )
                        else:
                            # partition APs from base 32 may span <=32 rows,
                            # so the l row needs its own copy
                            nc.vector.tensor_copy(oU[HD:128, :], oTs[1][HD:128, :])
                            nc.vector.tensor_copy(oU[32:33, :], oTs[1][32:33, :])

                    def c_bcast(parity=parity, lrow=lrow):
                        # broadcast l to ALL 128 partitions so the custom DVE
                        # reciprocal runs at partition base 0
                        lP = psP.tile(
                            [128, WT], F32, tag="lP", bufs=1,
                            name=f"lP{hcp}_{q}_{parity}",
                        )
                        lPs[parity] = lP
                        nc.tensor.matmul(
                            lP[:, :],
                            ones_sb[lrow : lrow + 1, :],
                            oUs[parity][lrow : lrow + 1, :],
                            start=True,
                            stop=True,
                        )

                    def c_recip(parity=parity):
                        rcb = pBuf.tile(
                            [128, WT], F32, tag="rcb", bufs=2,
                            name=f"rcb{hcp}_{q}_{parity}",
                        )
                        rcbs[parity] = rcb
                        if fast_recip:
                            nc.vector.reciprocal_approx_fast(rcb[:, :], lPs[parity][:, :])
                        else:
                            nc.vector.reciprocal(rcb[:, :], lPs[parity][:, :])

                    def c_mul(parity=parity):
                        ob = 0 if parity == 0 else HD
                        nc.vector.tensor_mul(
                            attnT_sb[ob : ob + HD, hcp, i0 : i0 + WT],
                            oUs[parity][ob : ob + HD, :],
                            rcbs[parity][ob : ob + HD, :],
                        )

                    return [(660, c_copy), (250, c_bcast), (690, c_recip), (600, c_mul)]

                yield from groups(0)
                ediv = div_chain(0)
                yield ediv[0]
                og = list(groups(1))
                rest = ediv[1:]
                for i, s in enumerate(og):
                    yield s
                    if i < len(rest):
                        yield rest[i]
                for j in range(len(og), len(rest)):
                    yield rest[j]
                for s in div_chain(1):
                    yield s

                if emit_wo:
                    for oc in range(OC):
                        def dstep(oc=oc):
                            ps_wo = psP.tile(
                                [128, WT], F32, tag="work", bufs=3,
                                name=f"pswo{q}_{oc}",
                            )
                            for dc in range(CC):
                                nc.tensor.matmul(
                                    ps_wo[:],
                                    wo_sb[:, dc, oc * 128 : (oc + 1) * 128],
                                    attnT_sb[:, dc, i0 : i0 + WT],
                                    start=(dc == 0),
                                    stop=(dc == CC - 1),
                                )
                            ot = pBuf.tile(
                                [128, WT], BF16, tag="ot", bufs=2,
                                name=f"ot{q}_{oc}",
                            )
                            if oc % 2 == 0:
                                nc.scalar.copy(ot[:], ps_wo[:])
                            else:
                                nc.vector.tensor_copy(ot[:], ps_wo[:])
                            nc.sync.dma_start(
                                yT.ap()[oc * 128 : (oc + 1) * 128, i0 : i0 + WT],
                                ot[:],
                            )

                        yield (900, dstep)

            def interleave(a_steps, b_steps):
                """Emit steps from both streams, pacing by estimated cost."""
                a, b = list(a_steps), list(b_steps)
                ta = sum(c for c, _ in a) or 1
                tb = sum(c for c, _ in b) or 1
                ca = cb = 0.0
                ai = bi = 0
                while ai < len(a) or bi < len(b):
                    if bi < len(b) and (ai >= len(a) or cb * ta < ca * tb):
                        cb += b[bi][0]
                        b[bi][1]()
                        bi += 1
                    else:
                        ca += a[ai][0]
                        a[ai][1]()
                        ai += 1

            # -------- emission schedule --------
            # A1: q/k for pair 0 (runs immediately; keeps the PE queue deep
            # enough that HAM stays warm through the pipeline ramp)
            for tw in range(TW):
                for _c, fn in qk_steps(0, tw):
                    fn()

            # A2: remaining projections, interleaved under pair-0 scores
            a2 = []
            for tj in range(4):
                a2.append(v_step(tj))
            for tw in range(TW):
                a2 += qk_steps(1, tw)
            for tj in range(4, 8):
                a2.append(v_step(tj))
            for tw in range(TW):
                a2 += qk_steps(2, tw)
            for tj in range(8, TC):
                a2.append(v_step(tj))

            # spread a2 across pair-0's four windows proportionally to the
            # score-stream cost, so the PE FIFO never head-of-line blocks
            # behind a score matmul that waits on exp
            sq_all = {
                (hcp, q): None for hcp in range(CC) for q in range(TW)
            }
            pT_tiles = {}
            for hcp in range(CC):
                for q in range(TW):
                    pT_e = pBuf.tile(
                        [128, PTW], BF16, tag="pT", bufs=4, name=f"pTe{hcp}_{q}"
                    )
                    pT_o = pBuf.tile(
                        [128, PTW], BF16, tag="pT", bufs=4, name=f"pTo{hcp}_{q}"
                    )
                    pT_tiles[(hcp, q)] = (pT_e, pT_o)

            sq0 = {q: list(sq_steps(0, q, *pT_tiles[(0, q)])) for q in range(TW)}
            sq0_costs = [sum(c for c, _ in sq0[q]) for q in range(TW)]
            total_sq0 = sum(sq0_costs)
            a2_total = sum(c for c, _ in a2)
            a2_slices = []
            acc = 0.0
            ai = 0
            for q in range(TW):
                acc += sq0_costs[q]
                target = a2_total * acc / total_sq0
                sl = []
                run = sum(c for c, _ in a2[:ai])
                while ai < len(a2) and run < target:
                    sl.append(a2[ai])
                    run += a2[ai][0]
                    ai += 1
                a2_slices.append(sl)
            a2_slices[-1] += a2[ai:]

            win_b = []
            for hcp in range(CC):
                for q in range(TW):
                    if hcp == 0:
                        # a2 slice BEFORE pv steps: emission order is the
                        # dependency contract (pv reads v_sb written by a2)
                        win_b = a2_slices[q] + win_b
                        sq = sq0[q]
                    else:
                        sq = list(sq_steps(hcp, q, *pT_tiles[(hcp, q)]))
                    interleave(sq, win_b)
                    win_b = list(
                        pv_steps(hcp, q, *pT_tiles[(hcp, q)], emit_wo=(hcp == CC - 1))
                    )
            for _c, fn in win_b:
                fn()

    nc.compile()
    return nc


def _get_nc():
    key = ("nc", CONFIG["fast_recip"])
    if key not in _CACHE:
        _CACHE[key] = _build_nc(CONFIG["fast_recip"])
    return _CACHE[key]


def kernel(x, W_qkv, b_qkv, W_o, b_o, **run_kwargs):
    x = np.asarray(x, dtype=np.float32)
    W_qkv = np.asarray(W_qkv, dtype=np.float32)
    b_qkv = np.asarray(b_qkv, dtype=np.float32)
    W_o = np.asarray(W_o, dtype=np.float32)
    b_o = np.asarray(b_o, dtype=np.float32)

    scale = np.float32(1.0) / np.sqrt(np.float32(HD)).astype(np.float32)

    in_maps = []
    for c in range(N_CORES):
        b = c // 2
        g = c % 2
        cs = g * CPC
        q_sl = slice(cs, cs + CPC)
        k_sl = slice(D + cs, D + cs + CPC)
        v_sl = slice(2 * D + cs, 2 * D + cs + CPC)
        in_maps.append(
            {
                "xT": np.ascontiguousarray(x[b].T).astype(BF16NP),
                "wq": (np.ascontiguousarray(W_qkv[:, q_sl]) * scale).astype(BF16NP),
                "wk": np.ascontiguousarray(W_qkv[:, k_sl]).astype(BF16NP),
                "wv": np.ascontiguousarray(W_qkv[:, v_sl]).astype(BF16NP),
                "bq": np.ascontiguousarray((b_qkv[q_sl] * scale).reshape(CC, 128).T),
                "bk": np.ascontiguousarray(b_qkv[k_sl].reshape(CC, 128).T),
                "bv": np.ascontiguousarray(np.broadcast_to(b_qkv[v_sl], (128, CPC))),
                "wo": np.ascontiguousarray(W_o[cs : cs + CPC, :]).astype(BF16NP),
            }
        )

    nc = _get_nc()
    res = run_bass_kernel_spmd(nc, in_maps, core_ids=list(range(N_CORES)), **run_kwargs)
    _CACHE["last_result"] = res

    out = np.empty((B, T, D), dtype=np.float32)
    for b in range(B):
        acc = res.results[2 * b]["yT"].astype(np.float32) + res.results[
            2 * b + 1
        ]["yT"].astype(np.float32)
        out[b] = acc.T + b_o
    return out


# revision 18
# speedup vs baseline: 1.1209x; 1.0150x over previous
import sys

if "/opt/trn_rl_repo" not in sys.path:
    sys.path.insert(0, "/opt/trn_rl_repo")

import numpy as np
import ml_dtypes

import concourse.bass as bass
import concourse.tile as tile
from concourse import bacc, mybir
from concourse.bass_utils import run_bass_kernel_spmd
from concourse.masks import make_upper_triangular

F32 = mybir.dt.float32
BF16 = mybir.dt.bfloat16
BF16NP = ml_dtypes.bfloat16
EXP = mybir.ActivationFunctionType.Exp

# Problem shape (hardcoded per contract)
B, T, D = 4, 2048, 768
H, HD = 12, 64
N_CORES = 8
HPC = 6                  # heads per core
CPC = HPC * HD           # 384 qkv columns per core
TC = T // 128            # 16 token blocks
DC = D // 128            # 6 chunks of model dim
CC = CPC // 128          # 3 head-pair chunks per core
OC = D // 128            # 6 output-col chunks
WT = 512                 # PSUM f32 bank width
TW = T // WT             # 4 query quarters

# Query-quarter geometry: quarter q covers queries i in [q*WT,(q+1)*WT),
# attends key blocks jc in 0..QJHI[q]; block jc contributes columns
# i in [max(jc*128, q*WT), (q+1)*WT) => width QW[q][jc]; stored densely
# at offset QOFF[q][jc] in the per-(head,quarter) pT tile.
QJHI = [min(4 * q + 3, TC - 1) for q in range(TW)]
QW = [[WT - max(0, jc * 128 - q * WT) for jc in range(QJHI[q] + 1)] for q in range(TW)]
QOFF = []
for _q in range(TW):
    _off = [0]
    for _w in QW[_q]:
        _off.append(_off[-1] + _w)
    QOFF.append(_off)
QCOLS = [QOFF[q][-1] for q in range(TW)]
PTW = max(QCOLS)         # 7424

_CACHE = {}
CONFIG = {"fast_recip": True}


def _build_nc(fast_recip=True):
    nc = bacc.Bacc("TRN2", target_bir_lowering=False, debug=False)

    xT = nc.dram_tensor("xT", [D, T], BF16, kind="ExternalInput")
    wq = nc.dram_tensor("wq", [D, CPC], BF16, kind="ExternalInput")
    wk = nc.dram_tensor("wk", [D, CPC], BF16, kind="ExternalInput")
    wv = nc.dram_tensor("wv", [D, CPC], BF16, kind="ExternalInput")
    bq = nc.dram_tensor("bq", [128, CC], F32, kind="ExternalInput")
    bk = nc.dram_tensor("bk", [128, CC], F32, kind="ExternalInput")
    bv = nc.dram_tensor("bv", [128, CPC], F32, kind="ExternalInput")
    wo = nc.dram_tensor("wo", [CPC, D], BF16, kind="ExternalInput")
    yT = nc.dram_tensor("yT", [D, T], BF16, kind="ExternalOutput")

    with tile.TileContext(nc) as tc:
        with (
            tc.tile_pool(name="persist", bufs=1) as pp,
            tc.tile_pool(name="loadA", bufs=1) as pA,
            tc.tile_pool(name="bufs", bufs=1) as pBuf,
            tc.tile_pool(name="ps", bufs=1, space="PSUM") as psP,
        ):
            # chunk hc holds head 2hc on partitions 0:64, head 2hc+1 on 64:128
            qT_sb = pp.tile([128, CC, T], BF16)
            kT_sb = pp.tile([128, CC, T], BF16)
            # v with denominator-ones column, positioned so PV output lands
            # on the partitions the head's attnT half needs directly:
            #   even head: [v(0:64) | ones@64 | ...]   -> out rows 0:64 + l@64
            #   odd head:  [0(:32) | ones@32 | 0 | v(64:128)] -> l@32 + rows 64:128
            v_sb = pp.tile([128, TC, HPC, 128], BF16)
            attnT_sb = pp.tile([128, CC, T], BF16)
            wo_sb = pp.tile([128, CC, D], BF16)
            bq_sb = pp.tile([128, CC], F32)
            bk_sb = pp.tile([128, CC], F32)
            bv_sb = pp.tile([128, HPC, HD], F32)
            maskf = pp.tile([128, 128], F32)
            mask01 = pp.tile([128, 128], BF16)
            ones_sb = pp.tile([128, 128], BF16)

            xT_sb = pA.tile([128, DC, T], BF16)
            wq_sb = pA.tile([128, DC, CPC], BF16)
            wk_sb = pA.tile([128, DC, CPC], BF16)
            wv_sb = pA.tile([128, DC, CPC], BF16)

            # score megas: one 1024-col (2-bank) ring per head of the pair;
            # pair alternation on ACT gives effective double-buffering.
            me = psP.tile([128, 1024], F32, tag="me", bufs=1)
            mo = psP.tile([128, 1024], F32, tag="mo", bufs=1)

            # -------- DMAs in need-order --------
            xT_r = xT.ap().rearrange("(o p) t -> p o t", p=128)
            wq_r = wq.ap().rearrange("(o p) c -> p o c", p=128)
            wk_r = wk.ap().rearrange("(o p) c -> p o c", p=128)
            wv_r = wv.ap().rearrange("(o p) c -> p o c", p=128)
            nc.sync.dma_start(xT_sb[:, :, 0:WT], xT_r[:, :, 0:WT])
            nc.sync.dma_start(wq_sb[:, :, 0:128], wq_r[:, :, 0:128])
            nc.sync.dma_start(wk_sb[:, :, 0:128], wk_r[:, :, 0:128])
            nc.sync.dma_start(bq_sb[:], bq.ap())
            nc.sync.dma_start(bk_sb[:], bk.ap())
            nc.sync.dma_start(xT_sb[:, :, WT : 2 * WT], xT_r[:, :, WT : 2 * WT])
            nc.sync.dma_start(wv_sb[:], wv_r[:])
            nc.sync.dma_start(bv_sb[:], bv.ap())
            nc.sync.dma_start(wq_sb[:, :, 128:CPC], wq_r[:, :, 128:CPC])
            nc.sync.dma_start(wk_sb[:, :, 128:CPC], wk_r[:, :, 128:CPC])
            nc.sync.dma_start(xT_sb[:, :, 2 * WT : 3 * WT], xT_r[:, :, 2 * WT : 3 * WT])
            nc.sync.dma_start(xT_sb[:, :, 3 * WT : T], xT_r[:, :, 3 * WT : T])
            nc.sync.dma_start(wo_sb[:], wo.ap().rearrange("(c p) o -> p c o", p=128))

            # -------- init masks / ones / v zeros (off the critical path) ----
            # mask01[j, i] = 1.0 if j <= i else 0.0 (valid causal region)
            make_upper_triangular(nc, maskf, val=1.0, diag=True)
            nc.vector.tensor_copy(mask01[:], maskf[:])
            nc.gpsimd.memset(ones_sb[:], 1.0)
            # odd-head lhsT cols 0:64 must be zero except the ones col at 32
            nc.gpsimd.memset(v_sb[:, :, 1:HPC:2, 0:HD], 0.0)
            nc.gpsimd.memset(v_sb[:, :, 0:HPC:2, HD : HD + 1], 1.0)
            nc.gpsimd.memset(v_sb[:, :, 1:HPC:2, 32:33], 1.0)

            # -------- Phase A steps --------
            def qk_steps(hc, tw):
                sp = slice(tw * WT, (tw + 1) * WT)
                cs = slice(hc * 128, (hc + 1) * 128)

                def q_step(hc=hc, tw=tw):
                    ps_q = psP.tile(
                        [128, WT], F32, tag="work", bufs=3, name=f"psq{hc}_{tw}"
                    )
                    for di in range(DC):
                        nc.tensor.matmul(
                            ps_q[:],
                            wq_sb[:, di, cs],
                            xT_sb[:, di, sp],
                            start=(di == 0),
                            stop=(di == DC - 1),
                        )
                    nc.vector.tensor_scalar_add(
                        qT_sb[:, hc, sp], ps_q[:], bq_sb[:, hc : hc + 1]
                    )

                def k_step(hc=hc, tw=tw):
                    ps_k = psP.tile(
                        [128, WT], F32, tag="work", bufs=3, name=f"psk{hc}_{tw}"
                    )
                    for di in range(DC):
                        nc.tensor.matmul(
                            ps_k[:],
                            wk_sb[:, di, cs],
                            xT_sb[:, di, sp],
                            start=(di == 0),
                            stop=(di == DC - 1),
                        )
                    nc.vector.tensor_scalar_add(
                        kT_sb[:, hc, sp], ps_k[:], bk_sb[:, hc : hc + 1]
                    )

                return [(1400, q_step), (1400, k_step)]

            def v_step(tj):
                def step(tj=tj):
                    ps_v = psP.tile(
                        [128, HPC, HD], F32, tag="work", bufs=3, name=f"psv{tj}"
                    )
                    for di in range(DC):
                        nc.tensor.matmul(
                            ps_v[:],
                            xT_sb[:, di, tj * 128 : (tj + 1) * 128],
                            wv_sb[:, di, :],
                            start=(di == 0),
                            stop=(di == DC - 1),
                        )
                    nc.vector.tensor_add(
                        v_sb[:, tj, 0:HPC:2, 0:HD],
                        ps_v[:, 0:HPC:2, :],
                        bv_sb[:, 0:HPC:2, :],
                    )
                    nc.vector.tensor_add(
                        v_sb[:, tj, 1:HPC:2, HD:128],
                        ps_v[:, 1:HPC:2, :],
                        bv_sb[:, 1:HPC:2, :],
                    )

                return (1200, step)

            # -------- Phase B: paired scores + exp for (pair, quarter) ------
            def sq_steps(hcp, q, pT_e, pT_o):
                i0 = q * WT
                jhi = QJHI[q]
                # dense segment list: (jc, abs_lo_i, n, dense_col)
                segs = []
                pcol = 0
                for jc in range(jhi + 1):
                    w = QW[q][jc]
                    lo = i0 + (WT - w)
                    s0 = 0
                    while s0 < w:
                        n = min(WT - (pcol % WT), w - s0)
                        segs.append((jc, lo + s0, n, pcol))
                        s0 += n
                        pcol += n
                total = pcol
                # diagonal-block mask positions (dense col of block start)
                mask_pos = [QOFF[q][jc] for jc in range(4 * q, jhi + 1)]
                chunks = []
                cur, cbase = [], 0
                for s in segs:
                    cur.append(s)
                    cend = s[3] + s[2]
                    if cend - cbase == 1024 or cend == total:
                        chunks.append((cbase, cend, cur))
                        cur, cbase = [], cend

                for cb, ce, cseg in chunks:
                    def step(cb=cb, ce=ce, cseg=cseg):
                        for jc, lo, n, pc in cseg:
                            rp = pc % 1024
                            kb = slice(jc * 128, (jc + 1) * 128)
                            # paired row-tiled K=64 matmuls: even head on array
                            # rows 0:64, odd on 64:128 -> run concurrently
                            nc.tensor.matmul(
                                me[:, rp : rp + n],
                                kT_sb[0:64, hcp, kb],
                                qT_sb[0:64, hcp, lo : lo + n],
                                start=True,
                                stop=True,
                            )
                            nc.tensor.matmul(
                                mo[:, rp : rp + n],
                                kT_sb[64:128, hcp, kb],
                                qT_sb[64:128, hcp, lo : lo + n],
                                start=True,
                                stop=True,
                            )
                        rb = cb % 1024
                        w = ce - cb
                        nc.scalar.activation(pT_e[:, cb:ce], me[:, rb : rb + w], EXP)
                        nc.scalar.activation(pT_o[:, cb:ce], mo[:, rb : rb + w], EXP)
                        for mp in mask_pos:
                            if cb < mp + 128 <= ce:
                                # diag-block masking on the otherwise-idle
                                # GPSIMD engine keeps the DVE free for the
                                # evac/div chains
                                nc.gpsimd.tensor_mul(
                                    pT_e[:, mp : mp + 128],
                                    pT_e[:, mp : mp + 128],
                                    mask01[:],
                                )
                                nc.gpsimd.tensor_mul(
                                    pT_o[:, mp : mp + 128],
                                    pT_o[:, mp : mp + 128],
                                    mask01[:],
                                )

                    yield (int((ce - cb) * 1.7) + 600, step)

            # -------- Phase B: PV + divide (+ W_o on last pair) -------------
            def pv_steps(hcp, q, pT_e, pT_o, emit_wo):
                i0 = q * WT
                jhi = QJHI[q]
                oTs, oUs, lPs, rcbs = {}, {}, {}, {}

                def groups(parity):
                    h = 2 * hcp + parity
                    pT = pT_e if parity == 0 else pT_o
                    mhi = HD + 1 if parity == 0 else 128
                    jcs = list(range(jhi + 1))
                    for g0 in range(0, len(jcs), 4):
                        grp = jcs[g0 : g0 + 4]

                        def step(grp=grp, g0=g0, h=h, pT=pT, mhi=mhi, parity=parity):
                            if g0 == 0:
                                oTs[parity] = psP.tile(
                                    [128, WT], F32, tag="work", bufs=3,
                                    name=f"oT{hcp}_{q}_{parity}",
                                )
                            oT = oTs[parity]
                            for jc in grp:
                                w = QW[q][jc]
                                nc.tensor.matmul(
                                    oT[0:mhi, WT - w : WT],
                                    v_sb[:, jc, h, 0:mhi],
                                    pT[:, QOFF[q][jc] : QOFF[q][jc] + w],
                                    start=(jc == 0),
                                    stop=(jc == jhi),
                                )

                        yield (len(grp) * 240, step)

                def div_chain(parity):
                    lrow = HD if parity == 0 else 32

                    def c_copy(parity=parity):
                        oU = pBuf.tile(
                            [128, WT], BF16, tag="oU", bufs=2,
                            name=f"oU{hcp}_{q}_{parity}",
                        )
                        oUs[parity] = oU
                        if parity == 0:
                            nc.vector.tensor_copy(
                                oU[0 : HD + 1, :], oTs[0][0 : HD + 1, :]
                            )
                        else:
                            # partition APs from base 32 may span <=32 rows,
                            # so the l row needs its own copy
                            nc.vector.tensor_copy(oU[HD:128, :], oTs[1][HD:128, :])
                            nc.vector.tensor_copy(oU[32:33, :], oTs[1][32:33, :])

                    def c_bcast(parity=parity, lrow=lrow):
                        # broadcast l to ALL 128 partitions so the custom DVE
                        # reciprocal runs at partition base 0
                        lP = psP.tile(
                            [128, WT], F32, tag="lP", bufs=1,
                            name=f"lP{hcp}_{q}_{parity}",
                        )
                        lPs[parity] = lP
                        nc.tensor.matmul(
                            lP[:, :],
                            ones_sb[lrow : lrow + 1, :],
                            oUs[parity][lrow : lrow + 1, :],
                            start=True,
                            stop=True,
                        )

                    def c_recip(parity=parity):
                        rcb = pBuf.tile(
                            [128, WT], F32, tag="rcb", bufs=2,
                            name=f"rcb{hcp}_{q}_{parity}",
                        )
                        rcbs[parity] = rcb
                        if fast_recip:
                            nc.vector.reciprocal_approx_fast(rcb[:, :], lPs[parity][:, :])
                        else:
                            nc.vector.reciprocal(rcb[:, :], lPs[parity][:, :])

                    def c_mul(parity=parity):
                        ob = 0 if parity == 0 else HD
                        nc.vector.tensor_mul(
                            attnT_sb[ob : ob + HD, hcp, i0 : i0 + WT],
                            oUs[parity][ob : ob + HD, :],
                            rcbs[parity][ob : ob + HD, :],
                        )

                    return [(660, c_copy), (250, c_bcast), (690, c_recip), (600, c_mul)]

                yield from groups(0)
                ediv = div_chain(0)
                yield ediv[0]
                og = list(groups(1))
                rest = ediv[1:]
                for i, s in enumerate(og):
                    yield s
                    if i < len(rest):
                        yield rest[i]
                for j in range(len(og), len(rest)):
                    yield rest[j]
                for s in div_chain(1):
                    yield s

                if emit_wo:
                    for oc in range(OC):
                        def dstep(oc=oc):
                            ps_wo = psP.tile(
                                [128, WT], F32, tag="work", bufs=3,
                                name=f"pswo{q}_{oc}",
                            )
                            for dc in range(CC):
                                nc.tensor.matmul(
                                    ps_wo[:],
                                    wo_sb[:, dc, oc * 128 : (oc + 1) * 128],
                                    attnT_sb[:, dc, i0 : i0 + WT],
                                    start=(dc == 0),
                                    stop=(dc == CC - 1),
                                )
                            ot = pBuf.tile(
                                [128, WT], BF16, tag="ot", bufs=2,
                                name=f"ot{q}_{oc}",
                            )
                            if oc % 2 == 0:
                                nc.scalar.copy(ot[:], ps_wo[:])
                            else:
                                nc.vector.tensor_copy(ot[:], ps_wo[:])
                            nc.sync.dma_start(
                                yT.ap()[oc * 128 : (oc + 1) * 128, i0 : i0 + WT],
                                ot[:],
                            )

                        yield (900, dstep)

            def interleave(a_steps, b_steps):
                """Emit steps from both streams, pacing by estimated cost."""
                a, b = list(a_steps), list(b_steps)
                ta = sum(c for c, _ in a) or 1
                tb = sum(c for c, _ in b) or 1
                ca = cb = 0.0
                ai = bi = 0
                while ai < len(a) or bi < len(b):
                    if bi < len(b) and (ai >= len(a) or cb * ta < ca * tb):
                        cb += b[bi][0]
                        b[bi][1]()
                        bi += 1
                    else:
                        ca += a[ai][0]
                        a[ai][1]()
                        ai += 1

            # -------- emission schedule --------
            # A1: q/k for pair 0 (runs immediately; keeps the PE queue deep
            # enough that HAM stays warm through the pipeline ramp)
            for tw in range(TW):
                for _c, fn in qk_steps(0, tw):
                    fn()

            # A2: remaining projections, interleaved under pair-0 scores
            a2 = []
            for tj in range(4):
                a2.append(v_step(tj))
            for tw in range(TW):
                a2 += qk_steps(1, tw)
            for tj in range(4, 8):
                a2.append(v_step(tj))
            for tw in range(TW):
                a2 += qk_steps(2, tw)
            for tj in range(8, TC):
                a2.append(v_step(tj))

            # spread a2 across pair-0's four windows proportionally to the
            # score-stream cost, so the PE FIFO never head-of-line blocks
            # behind a score matmul that waits on exp
            pT_tiles = {}
            for hcp in range(CC):
                for q in range(TW):
                    pT_e = pBuf.tile(
                        [128, PTW], BF16, tag="pT", bufs=4, name=f"pTe{hcp}_{q}"
                    )
                    pT_o = pBuf.tile(
                        [128, PTW], BF16, tag="pT", bufs=4, name=f"pTo{hcp}_{q}"
                    )
                    pT_tiles[(hcp, q)] = (pT_e, pT_o)

            sq0 = {q: list(sq_steps(0, q, *pT_tiles[(0, q)])) for q in range(TW)}
            sq0_costs = [sum(c for c, _ in sq0[q]) for q in range(TW)]
            total_sq0 = sum(sq0_costs)
            a2_total = sum(c for c, _ in a2)
            a2_slices = []
            acc = 0.0
            ai = 0
            for q in range(TW):
                acc += sq0_costs[q]
                target = a2_total * acc / total_sq0
                sl = []
                run = sum(c for c, _ in a2[:ai])
                while ai < len(a2) and run < target:
                    sl.append(a2[ai])
                    run += a2[ai][0]
                    ai += 1
                a2_slices.append(sl)
            a2_slices[-1] += a2[ai:]

            win_b = []
            for hcp in range(CC):
                for q in range(TW):
                    if hcp == 0:
                        # a2 slice BEFORE pv steps: emission order is the
                        # dependency contract (pv reads v_sb written by a2)
                        win_b = a2_slices[q] + win_b
                        sq = sq0[q]
                    else:
                        sq = list(sq_steps(hcp, q, *pT_tiles[(hcp, q)]))
                    interleave(sq, win_b)
                    win_b = list(
                        pv_steps(hcp, q, *pT_tiles[(hcp, q)], emit_wo=(hcp == CC - 1))
                    )
            for _c, fn in win_b:
                fn()

    nc.compile()
    return nc


def _get_nc():
    key = ("nc", CONFIG["fast_recip"])
    if key not in _CACHE:
        _CACHE[key] = _build_nc(CONFIG["fast_recip"])
    return _CACHE[key]


def kernel(x, W_qkv, b_qkv, W_o, b_o, **run_kwargs):
    x = np.asarray(x, dtype=np.float32)
    W_qkv = np.asarray(W_qkv, dtype=np.float32)
    b_qkv = np.asarray(b_qkv, dtype=np.float32)
    W_o = np.asarray(W_o, dtype=np.float32)
    b_o = np.asarray(b_o, dtype=np.float32)

    scale = np.float32(1.0) / np.sqrt(np.float32(HD)).astype(np.float32)

    in_maps = []
    for c in range(N_CORES):
        b = c // 2
        g = c % 2
        cs = g * CPC
        q_sl = slice(cs, cs + CPC)
        k_sl = slice(D + cs, D + cs + CPC)
        v_sl = slice(2 * D + cs, 2 * D + cs + CPC)
        in_maps.append(
            {
                "xT": np.ascontiguousarray(x[b].T).astype(BF16NP),
                "wq": (np.ascontiguousarray(W_qkv[:, q_sl]) * scale).astype(BF16NP),
                "wk": np.ascontiguousarray(W_qkv[:, k_sl]).astype(BF16NP),
                "wv": np.ascontiguousarray(W_qkv[:, v_sl]).astype(BF16NP),
                "bq": np.ascontiguousarray((b_qkv[q_sl] * scale).reshape(CC, 128).T),
                "bk": np.ascontiguousarray(b_qkv[k_sl].reshape(CC, 128).T),
                "bv": np.ascontiguousarray(np.broadcast_to(b_qkv[v_sl], (128, CPC))),
                "wo": np.ascontiguousarray(W_o[cs : cs + CPC, :]).astype(BF16NP),
            }
        )

    nc = _get_nc()
    res = run_bass_kernel_spmd(nc, in_maps, core_ids=list(range(N_CORES)), **run_kwargs)
    _CACHE["last_result"] = res

    out = np.empty((B, T, D), dtype=np.float32)
    for b in range(B):
        acc = res.results[2 * b]["yT"].astype(np.float32) + res.results[
            2 * b + 1
        ]["yT"].astype(np.float32)
        out[b] = acc.T + b_o
    return out


# revision 19
# speedup vs baseline: 1.1331x; 1.0108x over previous
import sys

if "/opt/trn_rl_repo" not in sys.path:
    sys.path.insert(0, "/opt/trn_rl_repo")

import numpy as np
import ml_dtypes

import concourse.bass as bass
import concourse.tile as tile
from concourse import bacc, mybir
from concourse.bass_utils import run_bass_kernel_spmd
from concourse.masks import make_upper_triangular

F32 = mybir.dt.float32
BF16 = mybir.dt.bfloat16
BF16NP = ml_dtypes.bfloat16
EXP = mybir.ActivationFunctionType.Exp

# Problem shape (hardcoded per contract)
B, T, D = 4, 2048, 768
H, HD = 12, 64
N_CORES = 8
HPC = 6                  # heads per core
CPC = HPC * HD           # 384 qkv columns per core
TC = T // 128            # 16 token blocks
DC = D // 128            # 6 chunks of model dim
CC = CPC // 128          # 3 head-pair chunks per core
OC = D // 128            # 6 output-col chunks
WT = 512                 # PSUM f32 bank width
TW = T // WT             # 4 query quarters

# Query-quarter geometry: quarter q covers queries i in [q*WT,(q+1)*WT),
# attends key blocks jc in 0..QJHI[q]; block jc contributes columns
# i in [max(jc*128, q*WT), (q+1)*WT) => width QW[q][jc]; stored densely
# at offset QOFF[q][jc] in the per-(head,quarter) pT tile.
QJHI = [min(4 * q + 3, TC - 1) for q in range(TW)]
QW = [[WT - max(0, jc * 128 - q * WT) for jc in range(QJHI[q] + 1)] for q in range(TW)]
QOFF = []
for _q in range(TW):
    _off = [0]
    for _w in QW[_q]:
        _off.append(_off[-1] + _w)
    QOFF.append(_off)
QCOLS = [QOFF[q][-1] for q in range(TW)]
PTW = max(QCOLS)         # 7424

_CACHE = {}
CONFIG = {"fast_recip": True}


def _build_nc(fast_recip=True):
    nc = bacc.Bacc("TRN2", target_bir_lowering=False, debug=False)

    xT = nc.dram_tensor("xT", [D, T], BF16, kind="ExternalInput")
    wq = nc.dram_tensor("wq", [D, CPC], BF16, kind="ExternalInput")
    wk = nc.dram_tensor("wk", [D, CPC], BF16, kind="ExternalInput")
    wv = nc.dram_tensor("wv", [D, CPC], BF16, kind="ExternalInput")
    bq = nc.dram_tensor("bq", [128, CC], F32, kind="ExternalInput")
    bk = nc.dram_tensor("bk", [128, CC], F32, kind="ExternalInput")
    bv = nc.dram_tensor("bv", [128, CPC], F32, kind="ExternalInput")
    wo = nc.dram_tensor("wo", [CPC, D], BF16, kind="ExternalInput")
    yT = nc.dram_tensor("yT", [D, T], BF16, kind="ExternalOutput")

    with tile.TileContext(nc) as tc:
        with (
            tc.tile_pool(name="persist", bufs=1) as pp,
            tc.tile_pool(name="loadA", bufs=1) as pA,
            tc.tile_pool(name="bufs", bufs=1) as pBuf,
            tc.tile_pool(name="ps", bufs=1, space="PSUM") as psP,
        ):
            # chunk hc holds head 2hc on partitions 0:64, head 2hc+1 on 64:128
            qT_sb = pp.tile([128, CC, T], BF16)
            kT_sb = pp.tile([128, CC, T], BF16)
            # v with denominator-ones column, positioned so PV output lands
            # on the partitions the head's attnT half needs directly:
            #   even head: [v(0:64) | ones@64 | ...]   -> out rows 0:64 + l@64
            #   odd head:  [0(:32) | ones@32 | 0 | v(64:128)] -> l@32 + rows 64:128
            v_sb = pp.tile([128, TC, HPC, 128], BF16)
            attnT_sb = pp.tile([128, CC, T], BF16)
            wo_sb = pp.tile([128, CC, D], BF16)
            bq_sb = pp.tile([128, CC], F32)
            bk_sb = pp.tile([128, CC], F32)
            bv_sb = pp.tile([128, HPC, HD], F32)
            maskf = pp.tile([128, 128], F32)
            mask01 = pp.tile([128, 128], BF16)
            ones_sb = pp.tile([128, 128], BF16)

            xT_sb = pA.tile([128, DC, T], BF16)
            wq_sb = pA.tile([128, DC, CPC], BF16)
            wk_sb = pA.tile([128, DC, CPC], BF16)
            wv_sb = pA.tile([128, DC, CPC], BF16)

            # score megas: one 1024-col (2-bank) ring per head of the pair;
            # pair alternation on ACT gives effective double-buffering.
            me = psP.tile([128, 1024], F32, tag="me", bufs=1)
            mo = psP.tile([128, 1024], F32, tag="mo", bufs=1)

            # -------- DMAs in need-order --------
            xT_r = xT.ap().rearrange("(o p) t -> p o t", p=128)
            wq_r = wq.ap().rearrange("(o p) c -> p o c", p=128)
            wk_r = wk.ap().rearrange("(o p) c -> p o c", p=128)
            wv_r = wv.ap().rearrange("(o p) c -> p o c", p=128)
            nc.sync.dma_start(xT_sb[:, :, 0:256], xT_r[:, :, 0:256])
            nc.sync.dma_start(xT_sb[:, :, 256:WT], xT_r[:, :, 256:WT])
            nc.sync.dma_start(wq_sb[:, :, 0:128], wq_r[:, :, 0:128])
            nc.sync.dma_start(wk_sb[:, :, 0:128], wk_r[:, :, 0:128])
            nc.sync.dma_start(bq_sb[:], bq.ap())
            nc.sync.dma_start(bk_sb[:], bk.ap())
            nc.sync.dma_start(xT_sb[:, :, WT : 2 * WT], xT_r[:, :, WT : 2 * WT])
            nc.sync.dma_start(wv_sb[:], wv_r[:])
            nc.sync.dma_start(bv_sb[:], bv.ap())
            nc.sync.dma_start(wq_sb[:, :, 128:CPC], wq_r[:, :, 128:CPC])
            nc.sync.dma_start(wk_sb[:, :, 128:CPC], wk_r[:, :, 128:CPC])
            nc.sync.dma_start(xT_sb[:, :, 2 * WT : 3 * WT], xT_r[:, :, 2 * WT : 3 * WT])
            nc.sync.dma_start(xT_sb[:, :, 3 * WT : T], xT_r[:, :, 3 * WT : T])
            nc.sync.dma_start(wo_sb[:], wo.ap().rearrange("(c p) o -> p c o", p=128))

            # -------- init masks / ones / v zeros (off the critical path) ----
            # mask01[j, i] = 1.0 if j <= i else 0.0 (valid causal region)
            make_upper_triangular(nc, maskf, val=1.0, diag=True)
            nc.vector.tensor_copy(mask01[:], maskf[:])
            nc.gpsimd.memset(ones_sb[:], 1.0)
            # odd-head lhsT cols 0:64 must be zero except the ones col at 32
            nc.gpsimd.memset(v_sb[:, :, 1:HPC:2, 0:HD], 0.0)
            nc.gpsimd.memset(v_sb[:, :, 0:HPC:2, HD : HD + 1], 1.0)
            nc.gpsimd.memset(v_sb[:, :, 1:HPC:2, 32:33], 1.0)

            # -------- Phase A steps --------
            def qk_steps(hc, tw):
                sp = slice(tw * WT, (tw + 1) * WT)
                cs = slice(hc * 128, (hc + 1) * 128)

                def q_step(hc=hc, tw=tw):
                    ps_q = psP.tile(
                        [128, WT], F32, tag="work", bufs=3, name=f"psq{hc}_{tw}"
                    )
                    for di in range(DC):
                        nc.tensor.matmul(
                            ps_q[:],
                            wq_sb[:, di, cs],
                            xT_sb[:, di, sp],
                            start=(di == 0),
                            stop=(di == DC - 1),
                        )
                    nc.vector.tensor_scalar_add(
                        qT_sb[:, hc, sp], ps_q[:], bq_sb[:, hc : hc + 1]
                    )

                def k_step(hc=hc, tw=tw):
                    ps_k = psP.tile(
                        [128, WT], F32, tag="work", bufs=3, name=f"psk{hc}_{tw}"
                    )
                    for di in range(DC):
                        nc.tensor.matmul(
                            ps_k[:],
                            wk_sb[:, di, cs],
                            xT_sb[:, di, sp],
                            start=(di == 0),
                            stop=(di == DC - 1),
                        )
                    nc.vector.tensor_scalar_add(
                        kT_sb[:, hc, sp], ps_k[:], bk_sb[:, hc : hc + 1]
                    )

                return [(1400, q_step), (1400, k_step)]

            def v_step(tj):
                def step(tj=tj):
                    ps_v = psP.tile(
                        [128, HPC, HD], F32, tag="work", bufs=3, name=f"psv{tj}"
                    )
                    for di in range(DC):
                        nc.tensor.matmul(
                            ps_v[:],
                            xT_sb[:, di, tj * 128 : (tj + 1) * 128],
                            wv_sb[:, di, :],
                            start=(di == 0),
                            stop=(di == DC - 1),
                        )
                    nc.vector.tensor_add(
                        v_sb[:, tj, 0:HPC:2, 0:HD],
                        ps_v[:, 0:HPC:2, :],
                        bv_sb[:, 0:HPC:2, :],
                    )
                    nc.vector.tensor_add(
                        v_sb[:, tj, 1:HPC:2, HD:128],
                        ps_v[:, 1:HPC:2, :],
                        bv_sb[:, 1:HPC:2, :],
                    )

                return (1200, step)

            # -------- Phase B: paired scores + exp for (pair, quarter) ------
            def sq_steps(hcp, q, pT_e, pT_o):
                i0 = q * WT
                jhi = QJHI[q]
                # dense segment list: (jc, abs_lo_i, n, dense_col)
                segs = []
                pcol = 0
                for jc in range(jhi + 1):
                    w = QW[q][jc]
                    lo = i0 + (WT - w)
                    s0 = 0
                    while s0 < w:
                        n = min(WT - (pcol % WT), w - s0)
                        segs.append((jc, lo + s0, n, pcol))
                        s0 += n
                        pcol += n
                total = pcol
                # diagonal-block mask positions (dense col of block start)
                mask_pos = [QOFF[q][jc] for jc in range(4 * q, jhi + 1)]
                chunks = []
                cur, cbase = [], 0
                for s in segs:
                    cur.append(s)
                    cend = s[3] + s[2]
                    if cend - cbase == 1024 or cend == total:
                        chunks.append((cbase, cend, cur))
                        cur, cbase = [], cend

                for cb, ce, cseg in chunks:
                    def step(cb=cb, ce=ce, cseg=cseg):
                        for jc, lo, n, pc in cseg:
                            rp = pc % 1024
                            kb = slice(jc * 128, (jc + 1) * 128)
                            # paired row-tiled K=64 matmuls: even head on array
                            # rows 0:64, odd on 64:128 -> run concurrently
                            nc.tensor.matmul(
                                me[:, rp : rp + n],
                                kT_sb[0:64, hcp, kb],
                                qT_sb[0:64, hcp, lo : lo + n],
                                start=True,
                                stop=True,
                            )
                            nc.tensor.matmul(
                                mo[:, rp : rp + n],
                                kT_sb[64:128, hcp, kb],
                                qT_sb[64:128, hcp, lo : lo + n],
                                start=True,
                                stop=True,
                            )
                        rb = cb % 1024
                        w = ce - cb
                        nc.scalar.activation(pT_e[:, cb:ce], me[:, rb : rb + w], EXP)
                        nc.scalar.activation(pT_o[:, cb:ce], mo[:, rb : rb + w], EXP)
                        for mp in mask_pos:
                            if cb < mp + 128 <= ce:
                                # diag-block masking on the otherwise-idle
                                # GPSIMD engine keeps the DVE free for the
                                # evac/div chains
                                nc.gpsimd.tensor_mul(
                                    pT_e[:, mp : mp + 128],
                                    pT_e[:, mp : mp + 128],
                                    mask01[:],
                                )
                                nc.gpsimd.tensor_mul(
                                    pT_o[:, mp : mp + 128],
                                    pT_o[:, mp : mp + 128],
                                    mask01[:],
                                )

                    yield (int((ce - cb) * 1.7) + 600, step)

            # -------- Phase B: PV + divide (+ W_o on last pair) -------------
            def pv_steps(hcp, q, pT_e, pT_o, emit_wo):
                i0 = q * WT
                jhi = QJHI[q]
                oTs, oUs, lPs, rcbs = {}, {}, {}, {}

                def groups(parity):
                    h = 2 * hcp + parity
                    pT = pT_e if parity == 0 else pT_o
                    mhi = HD + 1 if parity == 0 else 128
                    jcs = list(range(jhi + 1))
                    for g0 in range(0, len(jcs), 4):
                        grp = jcs[g0 : g0 + 4]

                        def step(grp=grp, g0=g0, h=h, pT=pT, mhi=mhi, parity=parity):
                            if g0 == 0:
                                oTs[parity] = psP.tile(
                                    [128, WT], F32, tag="work", bufs=3,
                                    name=f"oT{hcp}_{q}_{parity}",
                                )
                            oT = oTs[parity]
                            for jc in grp:
                                w = QW[q][jc]
                                nc.tensor.matmul(
                                    oT[0:mhi, WT - w : WT],
                                    v_sb[:, jc, h, 0:mhi],
                                    pT[:, QOFF[q][jc] : QOFF[q][jc] + w],
                                    start=(jc == 0),
                                    stop=(jc == jhi),
                                )

                        yield (len(grp) * 240, step)

                def div_chain(parity):
                    lrow = HD if parity == 0 else 32

                    def c_copy(parity=parity):
                        oU = pBuf.tile(
                            [128, WT], BF16, tag="oU", bufs=2,
                            name=f"oU{hcp}_{q}_{parity}",
                        )
                        oUs[parity] = oU
                        if parity == 0:
                            nc.vector.tensor_copy(
                                oU[0 : HD + 1, :], oTs[0][0 : HD + 1, :]
                            )
                        else:
                            # partition APs from base 32 may span <=32 rows,
                            # so the l row needs its own copy
                            nc.vector.tensor_copy(oU[HD:128, :], oTs[1][HD:128, :])
                            nc.vector.tensor_copy(oU[32:33, :], oTs[1][32:33, :])

                    def c_bcast(parity=parity, lrow=lrow):
                        # broadcast l to ALL 128 partitions so the custom DVE
                        # reciprocal runs at partition base 0
                        lP = psP.tile(
                            [128, WT], F32, tag="lP", bufs=1,
                            name=f"lP{hcp}_{q}_{parity}",
                        )
                        lPs[parity] = lP
                        nc.tensor.matmul(
                            lP[:, :],
                            ones_sb[lrow : lrow + 1, :],
                            oUs[parity][lrow : lrow + 1, :],
                            start=True,
                            stop=True,
                        )

                    def c_recip(parity=parity):
                        rcb = pBuf.tile(
                            [128, WT], F32, tag="rcb", bufs=2,
                            name=f"rcb{hcp}_{q}_{parity}",
                        )
                        rcbs[parity] = rcb
                        if fast_recip:
                            nc.vector.reciprocal_approx_fast(rcb[:, :], lPs[parity][:, :])
                        else:
                            nc.vector.reciprocal(rcb[:, :], lPs[parity][:, :])

                    def c_mul(parity=parity):
                        ob = 0 if parity == 0 else HD
                        nc.vector.tensor_mul(
                            attnT_sb[ob : ob + HD, hcp, i0 : i0 + WT],
                            oUs[parity][ob : ob + HD, :],
                            rcbs[parity][ob : ob + HD, :],
                        )

                    return [(660, c_copy), (250, c_bcast), (690, c_recip), (600, c_mul)]

                yield from groups(0)
                ediv = div_chain(0)
                yield ediv[0]
                og = list(groups(1))
                rest = ediv[1:]
                for i, s in enumerate(og):
                    yield s
                    if i < len(rest):
                        yield rest[i]
                for j in range(len(og), len(rest)):
                    yield rest[j]
                for s in div_chain(1):
                    yield s

                if emit_wo:
                    for oc in range(OC):
                        def dstep(oc=oc):
                            ps_wo = psP.tile(
                                [128, WT], F32, tag="work", bufs=3,
                                name=f"pswo{q}_{oc}",
                            )
                            for dc in range(CC):
                                nc.tensor.matmul(
                                    ps_wo[:],
                                    wo_sb[:, dc, oc * 128 : (oc + 1) * 128],
                                    attnT_sb[:, dc, i0 : i0 + WT],
                                    start=(dc == 0),
                                    stop=(dc == CC - 1),
                                )
                            ot = pBuf.tile(
                                [128, WT], BF16, tag="ot", bufs=4,
                                name=f"ot{q}_{oc}",
                            )
                            if oc % 2 == 0:
                                nc.scalar.copy(ot[:], ps_wo[:])
                            else:
                                nc.vector.tensor_copy(ot[:], ps_wo[:])
                            nc.sync.dma_start(
                                yT.ap()[oc * 128 : (oc + 1) * 128, i0 : i0 + WT],
                                ot[:],
                            )

                        yield (900, dstep)

            def interleave(a_steps, b_steps):
                """Emit steps from both streams, pacing by estimated cost."""
                a, b = list(a_steps), list(b_steps)
                ta = sum(c for c, _ in a) or 1
                tb = sum(c for c, _ in b) or 1
                ca = cb = 0.0
                ai = bi = 0
                while ai < len(a) or bi < len(b):
                    if bi < len(b) and (ai >= len(a) or cb * ta < ca * tb):
                        cb += b[bi][0]
                        b[bi][1]()
                        bi += 1
                    else:
                        ca += a[ai][0]
                        a[ai][1]()
                        ai += 1

            # -------- emission schedule --------
            # A1: q/k for pair 0 (runs immediately; keeps the PE queue deep
            # enough that HAM stays warm through the pipeline ramp). The
            # tw=0 groups run as two 256-wide halves so the PE starts as
            # soon as the first half of x lands.
            def qk_halves(hc):
                cs = slice(hc * 128, (hc + 1) * 128)
                for w_sb, b_sb, dst in (
                    (wq_sb, bq_sb, qT_sb),
                    (wk_sb, bk_sb, kT_sb),
                ):
                    ps = psP.tile(
                        [128, WT], F32, tag="work", bufs=3,
                        name=f"psh{hc}_{0 if dst is qT_sb else 1}",
                    )
                    for half in range(2):
                        hs = slice(half * 256, (half + 1) * 256)
                        for di in range(DC):
                            nc.tensor.matmul(
                                ps[:, hs],
                                w_sb[:, di, cs],
                                xT_sb[:, di, hs],
                                start=(di == 0),
                                stop=(di == DC - 1),
                            )
                    nc.vector.tensor_scalar_add(
                        dst[:, hc, 0:WT], ps[:], b_sb[:, hc : hc + 1]
                    )

            qk_halves(0)
            for tw in range(1, TW):
                for _c, fn in qk_steps(0, tw):
                    fn()

            # A2: remaining projections, interleaved under pair-0 scores
            a2 = []
            for tj in range(4):
                a2.append(v_step(tj))
            for tw in range(TW):
                a2 += qk_steps(1, tw)
            for tj in range(4, 8):
                a2.append(v_step(tj))
            for tw in range(TW):
                a2 += qk_steps(2, tw)
            for tj in range(8, TC):
                a2.append(v_step(tj))

            # spread a2 across pair-0's four windows proportionally to the
            # score-stream cost, so the PE FIFO never head-of-line blocks
            # behind a score matmul that waits on exp
            pT_tiles = {}
            for hcp in range(CC):
                for q in range(TW):
                    pT_e = pBuf.tile(
                        [128, PTW], BF16, tag="pT", bufs=4, name=f"pTe{hcp}_{q}"
                    )
                    pT_o = pBuf.tile(
                        [128, PTW], BF16, tag="pT", bufs=4, name=f"pTo{hcp}_{q}"
                    )
                    pT_tiles[(hcp, q)] = (pT_e, pT_o)

            sq0 = {q: list(sq_steps(0, q, *pT_tiles[(0, q)])) for q in range(TW)}
            sq0_costs = [sum(c for c, _ in sq0[q]) for q in range(TW)]
            total_sq0 = sum(sq0_costs)
            a2_total = sum(c for c, _ in a2)
            a2_slices = []
            acc = 0.0
            ai = 0
            for q in range(TW):
                acc += sq0_costs[q]
                target = a2_total * acc / total_sq0
                sl = []
                run = sum(c for c, _ in a2[:ai])
                while ai < len(a2) and run < target:
                    sl.append(a2[ai])
                    run += a2[ai][0]
                    ai += 1
                a2_slices.append(sl)
            a2_slices[-1] += a2[ai:]

            win_b = []
            for hcp in range(CC):
                for q in range(TW):
                    if hcp == 0:
                        # a2 slice BEFORE pv steps: emission order is the
                        # dependency contract (pv reads v_sb written by a2)
                        win_b = a2_slices[q] + win_b
                        sq = sq0[q]
                    else:
                        sq = list(sq_steps(hcp, q, *pT_tiles[(hcp, q)]))
                    interleave(sq, win_b)
                    win_b = list(
                        pv_steps(hcp, q, *pT_tiles[(hcp, q)], emit_wo=(hcp == CC - 1))
                    )
            for _c, fn in win_b:
                fn()

    nc.compile()
    return nc


def _get_nc():
    key = ("nc", CONFIG["fast_recip"])
    if key not in _CACHE:
        _CACHE[key] = _build_nc(CONFIG["fast_recip"])
    return _CACHE[key]


def kernel(x, W_qkv, b_qkv, W_o, b_o, **run_kwargs):
    x = np.asarray(x, dtype=np.float32)
    W_qkv = np.asarray(W_qkv, dtype=np.float32)
    b_qkv = np.asarray(b_qkv, dtype=np.float32)
    W_o = np.asarray(W_o, dtype=np.float32)
    b_o = np.asarray(b_o, dtype=np.float32)

    scale = np.float32(1.0) / np.sqrt(np.float32(HD)).astype(np.float32)

    in_maps = []
    for c in range(N_CORES):
        b = c // 2
        g = c % 2
        cs = g * CPC
        q_sl = slice(cs, cs + CPC)
        k_sl = slice(D + cs, D + cs + CPC)
        v_sl = slice(2 * D + cs, 2 * D + cs + CPC)
        in_maps.append(
            {
                "xT": np.ascontiguousarray(x[b].T).astype(BF16NP),
                "wq": (np.ascontiguousarray(W_qkv[:, q_sl]) * scale).astype(BF16NP),
                "wk": np.ascontiguousarray(W_qkv[:, k_sl]).astype(BF16NP),
                "wv": np.ascontiguousarray(W_qkv[:, v_sl]).astype(BF16NP),
                "bq": np.ascontiguousarray((b_qkv[q_sl] * scale).reshape(CC, 128).T),
                "bk": np.ascontiguousarray(b_qkv[k_sl].reshape(CC, 128).T),
                "bv": np.ascontiguousarray(np.broadcast_to(b_qkv[v_sl], (128, CPC))),
                "wo": np.ascontiguousarray(W_o[cs : cs + CPC, :]).astype(BF16NP),
            }
        )

    nc = _get_nc()
    res = run_bass_kernel_spmd(nc, in_maps, core_ids=list(range(N_CORES)), **run_kwargs)
    _CACHE["last_result"] = res

    out = np.empty((B, T, D), dtype=np.float32)
    for b in range(B):
        acc = res.results[2 * b]["yT"].astype(np.float32) + res.results[
            2 * b + 1
        ]["yT"].astype(np.float32)
        out[b] = acc.T + b_o
    return out
